# revision 1
# baseline (speedup 1.0000x reference)
"""Trainium2 Bass kernel for nn_BlockConv (PointNet-style GNN block), 8 cores.

Algebraic core: msg_e = concat(x_src, pos_src-pos_dst) @ W + b
  = A[src] - C[dst], with A = concat(x,pos)@W + b (per-node table) and
  C = pos@W[-3:] (per-dst, constant within a segment). segment_max over
  dst therefore = (gather+max of A rows) - C[dst]. Pure memory problem.

Distribution: dst-sharded. Core k owns dst nodes [k*NLOC,(k+1)*NLOC);
node tensors and weights replicated; h exchanged with one AllGather
(transposed so conv2 needs no on-chip transposes); BN stats via tiny
AllReduduce of per-core partial sums.

Gather: dma_gather (int16 idx) from an HBM A-table (row n+1 = A[n],
row 0 = -BIG lo-dummy, rows N+1.. = -BIG incl. hi-dummy). Edges split
into lo/hi source windows to fit int16; per window, the core's nodes
are sorted by degree so pass k covers a slot prefix; DVE max chains
accumulate; a final HBM round-trip regathers both accumulators in node
order and maxes them.
"""
import sys
import numpy as np

if "/opt/trn_rl_repo" not in sys.path:
    sys.path.insert(0, "/opt/trn_rl_repo")

BIG_NEG = -1.0e30
EPS = 1e-5

FULL_CFG = dict(N=50000, E=800000, CIN=64, COUT=128, NC=8,
                LO_LIM=32768, R=50432, HI_DUMMY=50176)
MINI_CFG = dict(N=2048, E=16384, CIN=64, COUT=128, NC=8,
                LO_LIM=1024, R=2432, HI_DUMMY=2176)
MID_CFG = dict(N=16384, E=262144, CIN=64, COUT=128, NC=8,
               LO_LIM=8192, R=16768, HI_DUMMY=16512)


def _ceil(a, b):
    return (a + b - 1) // b


def _wrap16(ids):
    """flat int list (len % 128 == 0) -> [128, len//16] int16 wrapped:
    unwrapped[j] = g[j%16, j//16], replicated over the 8 core groups."""
    a = np.asarray(ids, np.int64)
    assert a.size % 128 == 0 and a.min() >= 0 and a.max() < 32768
    g = a.reshape(a.size // 16, 16).T.astype(np.int16)   # [16, L/16]
    return np.tile(g, (8, 1))                            # [128, L/16]


def host_prep(edge_index, pos, cfg):
    N, NC, LO_LIM = cfg["N"], cfg["NC"], cfg["LO_LIM"]
    NLOC = N // NC
    SLOC = _ceil(NLOC, 128)
    NSLOT = SLOC * 128
    src = np.asarray(edge_index[0], np.int64)
    dst = np.asarray(edge_index[1], np.int64)
    rows = src + 1
    core_of = dst // NLOC

    sides = [[], []]     # sides[0][c] = lo side of core c
    for c in range(NC):
        m = core_of == c
        s_rows = rows[m]
        d_loc = dst[m] - c * NLOC
        for si, sel in ((0, s_rows < LO_LIM), (1, s_rows >= LO_LIM)):
            s = s_rows[sel] - (0 if si == 0 else LO_LIM)
            d = d_loc[sel]
            deg = np.bincount(d, minlength=NSLOT)
            order = np.argsort(-deg, kind="stable")
            slot_of = np.empty(NSLOT, np.int64)
            slot_of[order] = np.arange(NSLOT)
            isort = np.argsort(d, kind="stable")
            starts = np.zeros(NSLOT + 1, np.int64)
            np.cumsum(deg, out=starts[1:])
            sides[si].append({"deg": deg, "order": order, "slot_of": slot_of,
                              "s_sorted": s[isort], "starts": starts,
                              "cnts": np.sort(deg)[::-1]})

    sched = []
    for si in range(2):
        Sk = []
        kmax = max(int(sd["cnts"][0]) for sd in sides[si])
        for k in range(kmax):
            cnt = max(int((sd["cnts"] > k).sum()) for sd in sides[si])
            if cnt == 0:
                break
            Sk.append(_ceil(cnt, 128))
        sched.append(Sk)

    j = np.arange(NSLOT)
    n_of_j = (j % 128) * SLOC + j // 128

    # interleaved to match the 512-node build blocks: column b*512+k*128+p
    # holds pos of node b*512+4p+k
    q = np.arange(_ceil(N, 512) * 512)
    node_q = np.minimum((q // 512) * 512 + (q % 128) * 4 + (q % 512) // 128, N - 1)
    posT = np.ascontiguousarray(np.asarray(pos)[node_q].T.astype(np.float32))

    per_core = []
    for c in range(NC):
        blocks = {0: [], 1: []}
        for si in range(2):
            sd = sides[si][c]
            dummy = 0 if si == 0 else cfg["HI_DUMMY"] - LO_LIM
            for k, S in enumerate(sched[si]):
                L = S * 128
                ids = np.full(L, dummy, np.int64)
                nsl = int((sd["cnts"] > k).sum())
                nodes = sd["order"][:nsl]
                ids[:nsl] = sd["s_sorted"][sd["starts"][nodes] + k]
                blocks[si].append(_wrap16(ids))
        gi_lo = (np.concatenate(blocks[0], axis=1) if blocks[0]
                 else np.zeros((128, 8), np.int16))
        gi_hi = (np.concatenate(blocks[1], axis=1) if blocks[1]
                 else np.zeros((128, 8), np.int16))
        mg_lo = _wrap16(sides[0][c]["slot_of"][n_of_j])
        mg_hi = _wrap16(sides[1][c]["slot_of"][n_of_j] + NSLOT)
        mg_sk = _wrap16(n_of_j)
        gnode = np.minimum(c * NLOC + n_of_j, N - 1)
        posm = np.ascontiguousarray(np.asarray(pos)[gnode].T.astype(np.float32))
        per_core.append({"gi_lo": gi_lo, "gi_hi": gi_hi, "mg_lo": mg_lo,
                         "mg_hi": mg_hi, "mg_skip": mg_sk, "posm": posm})

    # conv2 window pos permutation (global, replicated):
    win = np.arange(NC * SLOC)
    cols = []
    for w in win:
        ct, ww = w // SLOC, w % SLOC
        nodes = ct * NLOC + np.arange(128) * SLOC + ww
        cols.append(np.minimum(nodes, N - 1))
    posw = np.ascontiguousarray(
        np.asarray(pos)[np.concatenate(cols)].T.astype(np.float32))
    return per_core, (posw, posT), sched


def build_bass(cfg, sched, reps=1, timeline=False):
    import concourse.bass as bass
    import concourse.bacc as bacc
    import concourse.tile as tile
    from concourse import mybir
    from concourse.masks import make_identity
    import contextlib

    N, NC = cfg["N"], cfg["NC"]
    CIN, COUT = cfg["CIN"], cfg["COUT"]
    NLOC = N // NC
    SLOC = _ceil(NLOC, 128)
    NSLOT = SLOC * 128
    LO_LIM, R = cfg["LO_LIM"], cfg["R"]
    NCHUNK = _ceil(N, 128)
    NSK = _ceil(NLOC, 128)
    f32, i16 = mybir.dt.float32, mybir.dt.int16
    OP = mybir.AluOpType
    AX = mybir.AxisListType
    AF = mybir.ActivationFunctionType

    nc = bacc.Bacc(num_devices=(1 if timeline else NC), name="blockconv")

    x_in = nc.dram_tensor("x", [N, CIN], f32, kind="ExternalInput")
    pos_in = nc.dram_tensor("pos", [N, 3], f32, kind="ExternalInput")
    xs_in = nc.dram_tensor("xs", [NLOC, CIN], f32, kind="ExternalInput")
    posm_in = nc.dram_tensor("posm", [3, NSLOT], f32, kind="ExternalInput")
    posw_in = nc.dram_tensor("posw", [3, NC * NSLOT], f32, kind="ExternalInput")
    posT_in = nc.dram_tensor("posT", [3, _ceil(N, 512) * 512], f32, kind="ExternalInput")
    wt = {}
    for nm, shp in (("W1", [CIN + 3, COUT]), ("b1", [1, COUT]),
                    ("W2", [COUT + 3, COUT]), ("b2", [1, COUT]),
                    ("Wl", [CIN, COUT]), ("bl", [1, COUT]),
                    ("g1", [COUT, 1]), ("be1", [COUT, 1]), ("g2", [COUT, 1]),
                    ("be2", [COUT, 1]), ("gl", [COUT, 1]), ("bel", [COUT, 1])):
        wt[nm] = nc.dram_tensor(nm, shp, f32, kind="ExternalInput")

    Wlo = max(sum(sched[0]), 1) * 8
    Whi = max(sum(sched[1]), 1) * 8
    gi_lo_in = nc.dram_tensor("gi_lo", [128, Wlo], i16, kind="ExternalInput")
    gi_hi_in = nc.dram_tensor("gi_hi", [128, Whi], i16, kind="ExternalInput")
    mg_lo_in = nc.dram_tensor("mg_lo", [128, NSLOT // 16], i16, kind="ExternalInput")
    mg_hi_in = nc.dram_tensor("mg_hi", [128, NSLOT // 16], i16, kind="ExternalInput")
    mg_sk_in = nc.dram_tensor("mg_skip", [128, NSLOT // 16], i16, kind="ExternalInput")

    out_t = nc.dram_tensor("out", [NSLOT, COUT], f32, kind="ExternalOutput")

    HI_R = R - LO_LIM
    table_lo = nc.dram_tensor("atable_lo", [LO_LIM + SLOC * 128 + 128, COUT], f32)
    table_hi = nc.dram_tensor("atable_hi", [HI_R + SLOC * 128 + 128, COUT], f32)
    mbuf = nc.dram_tensor("mbuf", [2 * NSLOT, COUT], f32)
    skipb = nc.dram_tensor("skipbuf", [NSLOT, COUT], f32)
    ag_i = nc.dram_tensor("ag_in", [COUT, NSLOT], f32)
    ag_o = nc.dram_tensor("ag_out", [NC, COUT, NSLOT], f32, addr_space=("Local" if timeline else "Shared"))
    ar_i = nc.dram_tensor("ar_in", [COUT, 4], f32)
    ar_o = nc.dram_tensor("ar_out", [COUT, 4], f32, addr_space=("Local" if timeline else "Shared"))
    rowbuf = nc.dram_tensor("rowbuf", [6, COUT], f32)
    ar2_i = nc.dram_tensor("ar2_in", [COUT, 2], f32)
    ar2_o = nc.dram_tensor("ar2_out", [COUT, 2], f32, addr_space=("Local" if timeline else "Shared"))
    groups = [list(range(NC))]

    with tile.TileContext(nc) as tc:
        ctx = contextlib.ExitStack()
        with ctx:
            sing = ctx.enter_context(tc.tile_pool(name="sing", bufs=1))
            xp = ctx.enter_context(tc.tile_pool(name="xp", bufs=3))
            pp = ctx.enter_context(tc.tile_pool(name="pp", bufs=2, space="PSUM"))
            pq = ctx.enter_context(tc.tile_pool(name="pq", bufs=2, space="PSUM"))
            pr = ctx.enter_context(tc.tile_pool(name="pr", bufs=1, space="PSUM"))
            cp = ctx.enter_context(tc.tile_pool(name="cp", bufs=4))
            ap_ = ctx.enter_context(tc.tile_pool(name="ap", bufs=1))
            st = ctx.enter_context(tc.tile_pool(name="st", bufs=2))
            sm = ctx.enter_context(tc.tile_pool(name="sm", bufs=2))

            ident = sing.tile([128, 128], f32)
            make_identity(nc, ident)
            ones1 = sing.tile([1, 128], f32)
            nc.vector.memset(ones1[:], 1.0)
            onesp = sing.tile([128, 1], f32)
            nc.vector.memset(onesp[:], 1.0)
            negbig = sing.tile([128, COUT], f32)
            nc.vector.memset(negbig[:], BIG_NEG)
            epsv = sing.tile([COUT, 1], f32)
            nc.vector.memset(epsv[:], EPS)

            W1s = sing.tile([CIN + 3, COUT], f32)
            nc.sync.dma_start(W1s[:], wt["W1"][:])
            W1ps = sing.tile([3, COUT], f32)
            nc.sync.dma_start(W1ps[:], wt["W1"][CIN:CIN + 3, :])
            W2as = sing.tile([COUT, COUT], f32)
            nc.sync.dma_start(W2as[:], wt["W2"][0:COUT, :])
            W2ps = sing.tile([3, COUT], f32)
            nc.sync.dma_start(W2ps[:], wt["W2"][COUT:COUT + 3, :])
            Wls = sing.tile([CIN, COUT], f32)
            nc.sync.dma_start(Wls[:], wt["Wl"][:])
            brow = {}
            for nm in ("b1", "b2", "bl"):
                t = sing.tile([1, COUT], f32, tag=f"br_{nm}")
                nc.sync.dma_start(t[:], wt[nm][:])
                brow[nm] = t
            b1bc = sing.tile([128, COUT], f32)
            _b1ap = wt["b1"][:]
            nc.sync.dma_start(b1bc[:], bass.AP(tensor=_b1ap.tensor, offset=_b1ap.offset,
                                               ap=[[0, 128]] + list(_b1ap.ap[1:])))
            pvec = {}
            for nm in ("g1", "be1", "g2", "be2", "gl", "bel"):
                v = sing.tile([COUT, 1], f32, tag=f"pv_{nm}")
                nc.sync.dma_start(v[:], wt[nm][:])
                pvec[nm] = v

            idx_lo = sing.tile([128, Wlo], i16)
            nc.sync.dma_start(idx_lo[:], gi_lo_in[:])
            idx_hi = sing.tile([128, Whi], i16)
            nc.sync.dma_start(idx_hi[:], gi_hi_in[:])
            midx = {}
            for nm, t_ in (("lo", mg_lo_in), ("hi", mg_hi_in), ("sk", mg_sk_in)):
                m_ = sing.tile([128, NSLOT // 16], i16, tag=f"mi_{nm}")
                nc.sync.dma_start(m_[:], t_[:])
                midx[nm] = m_

            for _rep in range(reps):
                # -BIG rows: lo dummy row 0; hi rows N+1..R
                nc.sync.dma_start(table_lo[0:1, :], negbig[0:1, :])
                r = N + 1
                while r < R:
                    nn = min(128, R - r)
                    nc.sync.dma_start(table_hi[r - LO_LIM:r - LO_LIM + nn, :], negbig[0:nn, :])
                    r += nn

                def table_write(src_tile, base, nrows):
                    lo_n = max(0, min(LO_LIM - base, nrows))
                    if lo_n > 0:
                        nc.sync.dma_start(table_lo[base:base + lo_n, :], src_tile[0:lo_n, :])
                    if lo_n < nrows:
                        hb = base + lo_n - LO_LIM
                        nc.sync.dma_start(table_hi[hb:hb + nrows - lo_n, :],
                                          src_tile[lo_n:nrows, :])

                # ---------------- conv1 A-table build ----------------
                # 512-node blocks: partition p holds rows base+4p..+3 (1KB
                # contiguous per partition); 4 interleaved transposes; table
                # rows written with stride 4.
                def stride4_write(src_tile, A, mlim):
                    m0 = max(0, min(mlim, _ceil(LO_LIM - A, 4)))
                    if m0 > 0:
                        d = table_lo[A:A + m0 * 4, :].rearrange(
                            "(m s) f -> m s f", s=4)[:, 0, :]
                        nc.sync.dma_start(d, src_tile[0:m0, :])
                    if m0 < mlim:
                        b2 = A + m0 * 4 - LO_LIM
                        d = table_hi[b2:b2 + (mlim - m0) * 4, :].rearrange(
                            "(m s) f -> m s f", s=4)[:, 0, :]
                        nc.sync.dma_start(d, src_tile[m0:mlim, :])

                NBLK = _ceil(N, 512)
                for b in range(NBLK):
                    base = b * 512
                    nload = min(512, N - base)
                    pmax = nload // 4
                    xt4 = xp.tile([128, 4, CIN], f32, tag="xload")
                    nc.sync.dma_start(
                        xt4[:pmax],
                        x_in[base:base + nload, :].rearrange("(p r) c -> p r c", r=4))
                    for k in range(4):
                        mlim_k = max(0, min(128, _ceil(N - base - k, 4)))
                        if mlim_k == 0:
                            continue
                        ps = pp.tile([CIN, 128], f32, tag="pst")
                        nc.tensor.transpose(out=ps[:], in_=xt4[:, k, :], identity=ident[:])
                        lhs = xp.tile([CIN + 3, 128], f32, tag="lhs")
                        nc.sync.dma_start(lhs[CIN:CIN + 3, :],
                                          posT_in[:, b * 512 + k * 128:b * 512 + (k + 1) * 128])
                        nc.scalar.copy(out=lhs[0:CIN, :], in_=ps[:])
                        pb = pq.tile([128, COUT], f32, tag="pout")
                        nc.tensor.matmul(out=pb[:], lhsT=lhs[:], rhs=W1s[:], start=True, stop=False)
                        nc.tensor.matmul(out=pb[:], lhsT=ones1[:], rhs=brow["b1"][:], start=False, stop=True)
                        oc = cp.tile([128, COUT], f32, tag="oc")
                        nc.vector.tensor_copy(out=oc[:], in_=pb[:])
                        stride4_write(oc, 1 + base + k, mlim_k)

                # ---------------- skip path (x slice @ Wl + bl) ----------------
                sk_s = sm.tile([128, COUT], f32, tag="sk_s")
                sk_q = sm.tile([128, COUT], f32, tag="sk_q")
                nc.vector.memset(sk_s[:], 0.0)
                nc.vector.memset(sk_q[:], 0.0)
                for c in range(NSK):
                    r0 = c * 128
                    nrow = min(128, NLOC - r0)
                    xt = xp.tile([128, CIN], f32, tag="xload")
                    nc.sync.dma_start(xt[:nrow, :], xs_in[r0:r0 + nrow, :])
                    ps = pp.tile([CIN, 128], f32, tag="pst")
                    nc.tensor.transpose(out=ps[:], in_=xt[:], identity=ident[:])
                    lhs = xp.tile([CIN + 3, 128], f32, tag="lhs")
                    nc.scalar.copy(out=lhs[0:CIN, :], in_=ps[:])
                    pb = pq.tile([128, COUT], f32, tag="pout")
                    nc.tensor.matmul(out=pb[:], lhsT=lhs[0:CIN, :], rhs=Wls[:], start=True, stop=False)
                    nc.tensor.matmul(out=pb[:], lhsT=ones1[:], rhs=brow["bl"][:], start=False, stop=True)
                    oc = cp.tile([128, COUT], f32, tag="oc")
                    nc.vector.tensor_copy(out=oc[:], in_=pb[:])
                    nc.sync.dma_start(skipb[r0:r0 + 128, :], oc[:])
                    nc.vector.tensor_tensor(out=sk_s[:nrow, :], in0=sk_s[:nrow, :], in1=oc[:nrow, :], op=OP.add)
                    sq = cp.tile([128, COUT], f32, tag="sq")
                    nc.vector.tensor_tensor(out=sq[:nrow, :], in0=oc[:nrow, :], in1=oc[:nrow, :], op=OP.mult)
                    nc.vector.tensor_tensor(out=sk_q[:nrow, :], in0=sk_q[:nrow, :], in1=sq[:nrow, :], op=OP.add)

                # stats staging tile [COUT, 4]: cols 0,1 conv1 sum/sq; 2,3 skip
                arst = sing.tile([COUT, 4], f32)
                pss = pr.tile([COUT, 2], f32, tag="pstat")
                nc.tensor.matmul(out=pss[:, 0:1], lhsT=sk_s[:], rhs=onesp[:], start=True, stop=True)
                nc.tensor.matmul(out=pss[:, 1:2], lhsT=sk_q[:], rhs=onesp[:], start=True, stop=True)
                nc.vector.tensor_copy(out=arst[:, 2:4], in_=pss[:])

                # ---------------- gather-max passes ----------------
                GMAX = 8   # max 8*128=1024 indices per dma_gather (HW SWDGE ring cap)

                def gather_chunked(dst3, in_ap, idxt, chunk0, nchunks):
                    a = 0
                    while a < nchunks:
                        b = min(a + GMAX, nchunks)
                        nc.gpsimd.dma_gather(
                            out_ap=dst3[:, a:b, :], in_ap=in_ap,
                            idxs_ap=idxt[:, (chunk0 + a) * 8:(chunk0 + b) * 8],
                            num_idxs=(b - a) * 128, num_idxs_reg=(b - a) * 128,
                            elem_size=COUT)
                        a = b

                def gather_conv(conv_idx):
                    acc = {}
                    for snm in ("lo", "hi"):
                        a = ap_.tile([128, SLOC, COUT], f32, tag=f"acc_{snm}")
                        nc.gpsimd.memset(a[:], BIG_NEG)
                        acc[snm] = a
                    for snm, idxt, wtab, winsz in (
                            ("lo", idx_lo, table_lo, LO_LIM), ("hi", idx_hi, table_hi, HI_R)):
                        off = 0
                        for k, S in enumerate(sched[0 if snm == "lo" else 1]):
                            stg = st.tile([128, SLOC, COUT], f32, tag="stage")
                            gather_chunked(stg[:, 0:S, :], wtab[0:winsz, :],
                                           idxt, off // 8, S)
                            nc.vector.tensor_tensor(
                                out=acc[snm][:, 0:S, :], in0=acc[snm][:, 0:S, :],
                                in1=stg[:, 0:S, :], op=OP.max)
                            off += 8 * S
                    # merge via HBM round-trip, node order
                    nc.sync.dma_start(
                        mbuf[0:NSLOT, :].rearrange("(s p) f -> p s f", p=128), acc["lo"][:])
                    nc.sync.dma_start(
                        mbuf[NSLOT:2 * NSLOT, :].rearrange("(s p) f -> p s f", p=128), acc["hi"][:])
                    g1t = st.tile([128, SLOC, COUT], f32, tag="stage")
                    gather_chunked(g1t[:], mbuf[:], midx["lo"], 0, SLOC)
                    g2t = st.tile([128, SLOC, COUT], f32, tag="stage")
                    gather_chunked(g2t[:], mbuf[:], midx["hi"], 0, SLOC)
                    agg = ap_.tile([128, SLOC, COUT], f32, tag="acc_lo")
                    nc.vector.tensor_tensor(out=agg[:], in0=g1t[:], in1=g2t[:], op=OP.max)
                    return agg

                agg1 = gather_conv(1)

                # mask = (agg1 > -1e29): 1.0 / 0.0  (deg-0 slots; reused for conv2)
                mask = sing.tile([128, SLOC, COUT], f32)
                nc.vector.tensor_scalar(out=mask[:], in0=agg1[:], scalar1=-1.0e29,
                                        scalar2=None, op0=OP.is_gt)

                # v1 = (agg1 - (c1 - b1)) * mask, per chunk s
                v1 = ap_.tile([128, SLOC, COUT], f32, tag="acc_hi")
                for s in range(SLOC):
                    pm = xp.tile([3, 128], f32, tag="posm")
                    nc.sync.dma_start(pm[:], posm_in[:, s * 128:(s + 1) * 128])
                    pc = pq.tile([128, COUT], f32, tag="pout")
                    nc.tensor.matmul(out=pc[:], lhsT=pm[:], rhs=W1ps[:], start=True, stop=True)
                    cb = cp.tile([128, COUT], f32, tag="cb")
                    nc.vector.tensor_tensor(out=cb[:], in0=pc[:], in1=b1bc[:], op=OP.subtract)
                    t_ = cp.tile([128, COUT], f32, tag="tv")
                    nc.vector.tensor_tensor(out=t_[:], in0=agg1[:, s, :], in1=cb[:], op=OP.subtract)
                    nc.vector.tensor_tensor(out=v1[:, s, :], in0=t_[:], in1=mask[:, s, :], op=OP.mult)

                # conv1 stats over v1
                def stats_into(vtile, arcols):
                    red = sm.tile([128, COUT], f32, tag="red")
                    nc.vector.tensor_reduce(out=red[:], in_=vtile[:].rearrange("p s f -> p f s"),
                                            op=OP.add, axis=AX.X)
                    vsq = st.tile([128, SLOC, COUT], f32, tag="stage")
                    nc.vector.tensor_tensor(out=vsq[:], in0=vtile[:], in1=vtile[:], op=OP.mult)
                    redq = sm.tile([128, COUT], f32, tag="redq")
                    nc.vector.tensor_reduce(out=redq[:], in_=vsq[:].rearrange("p s f -> p f s"),
                                            op=OP.add, axis=AX.X)
                    pst_ = pr.tile([COUT, 2], f32, tag="pstat")
                    nc.tensor.matmul(out=pst_[:, 0:1], lhsT=red[:], rhs=onesp[:], start=True, stop=True)
                    nc.tensor.matmul(out=pst_[:, 1:2], lhsT=redq[:], rhs=onesp[:], start=True, stop=True)
                    nc.vector.tensor_copy(out=arcols, in_=pst_[:])

                stats_into(v1, arst[:, 0:2])
                nc.sync.dma_start(ar_i[:], arst[:])
                if timeline:
                    _t = sm.tile([COUT, 4], f32, tag="cc1")
                    nc.sync.dma_start(_t[:], ar_i[:])
                    nc.sync.dma_start(ar_o[:], _t[:])
                else:
                    nc.gpsimd.collective_compute("AllReduce", OP.add, replica_groups=groups,
                                                 ins=[ar_i[:]], outs=[ar_o[:]])
                arres = sing.tile([COUT, 4], f32, tag="arres")
                nc.sync.dma_start(arres[:], ar_o[:])

                # BN params: scale = g * rsqrt(var+eps), shift = be - mean*scale
                def bn_params(sum_ap, sq_ap, g_v, be_v, tagp):
                    mean = sm.tile([COUT, 1], f32, tag=f"{tagp}_m")
                    nc.vector.tensor_scalar(out=mean[:], in0=sum_ap, scalar1=1.0 / N,
                                            scalar2=None, op0=OP.mult)
                    ex2 = sm.tile([COUT, 1], f32, tag=f"{tagp}_e")
                    nc.vector.tensor_scalar(out=ex2[:], in0=sq_ap, scalar1=1.0 / N,
                                            scalar2=None, op0=OP.mult)
                    m2 = sm.tile([COUT, 1], f32, tag=f"{tagp}_m2")
                    nc.vector.tensor_tensor(out=m2[:], in0=mean[:], in1=mean[:], op=OP.mult)
                    var = sm.tile([COUT, 1], f32, tag=f"{tagp}_v")
                    nc.vector.tensor_tensor(out=var[:], in0=ex2[:], in1=m2[:], op=OP.subtract)
                    sd = sm.tile([COUT, 1], f32, tag=f"{tagp}_sd")
                    nc.scalar.activation(out=sd[:], in_=var[:], func=AF.Sqrt, bias=epsv[:], scale=1.0)
                    rstd = sm.tile([COUT, 1], f32, tag=f"{tagp}_r")
                    nc.vector.reciprocal(out=rstd[:], in_=sd[:])
                    ssh = sm.tile([COUT, 2], f32, tag=f"{tagp}_ssh")
                    nc.vector.tensor_tensor(out=ssh[:, 0:1], in0=rstd[:], in1=g_v[:], op=OP.mult)
                    ms = sm.tile([COUT, 1], f32, tag=f"{tagp}_ms")
                    nc.vector.tensor_tensor(out=ms[:], in0=mean[:], in1=ssh[:, 0:1], op=OP.mult)
                    nc.vector.tensor_tensor(out=ssh[:, 1:2], in0=be_v[:], in1=ms[:], op=OP.subtract)
                    # transpose [COUT,2] -> [2, COUT] rows (scale row 0, shift row 1)
                    prow = pr.tile([2, COUT], f32, tag="prow")
                    nc.tensor.transpose(out=prow[:], in_=ssh[:], identity=ident[:])
                    rows = sing.tile([2, COUT], f32, tag=f"{tagp}_rows")
                    nc.vector.tensor_copy(out=rows[:], in_=prow[:])
                    slot = {"bn1": 0, "bnl": 2, "bn2": 4}[tagp]
                    nc.sync.dma_start(rowbuf[slot:slot + 2, :], rows[:])
                    bc = sing.tile([128, 2, COUT], f32, tag=f"{tagp}_bc")
                    rap = rowbuf[slot:slot + 2, :]
                    nc.sync.dma_start(bc[:], bass.AP(tensor=rap.tensor, offset=rap.offset,
                                                     ap=[[0, 128]] + list(rap.ap)))
                    return bc

                rows1 = bn_params(arres[:, 0:1], arres[:, 1:2], pvec["g1"], pvec["be1"], "bn1")
                rowsl = bn_params(arres[:, 2:3], arres[:, 3:4], pvec["gl"], pvec["bel"], "bnl")

                # h = relu(v1*scale1 + shift1); build transposed hT chunks -> ag_in
                h1 = ap_.tile([128, SLOC, COUT], f32, tag="acc_hi2")
                sc3 = rows1[:, 0:1, :].to_broadcast([128, SLOC, COUT])
                sh3 = rows1[:, 1:2, :].to_broadcast([128, SLOC, COUT])
                nc.vector.tensor_tensor(out=h1[:], in0=v1[:], in1=sc3, op=OP.mult)
                nc.vector.tensor_tensor(out=h1[:], in0=h1[:], in1=sh3, op=OP.add)
                nc.vector.tensor_scalar(out=h1[:], in0=h1[:], scalar1=0.0, scalar2=None, op0=OP.max)
                for s in range(SLOC):
                    ph = pq.tile([128, 128], f32, tag="pout")
                    nc.tensor.transpose(out=ph[:], in_=h1[:, s, :], identity=ident[:])
                    hc = cp.tile([128, 128], f32, tag="oc")
                    nc.scalar.copy(out=hc[:], in_=ph[:])
                    nc.sync.dma_start(ag_i[:, s * 128:(s + 1) * 128], hc[:])

                if timeline:
                    for _s in range(SLOC):
                        _t = cp.tile([128, 128], f32, tag="oc")
                        nc.sync.dma_start(_t[:], ag_i[:, _s * 128:(_s + 1) * 128])
                        nc.sync.dma_start(ag_o[0, :, _s * 128:(_s + 1) * 128], _t[:])
                else:
                    nc.gpsimd.collective_compute("AllGather", OP.bypass, replica_groups=groups,
                                                 ins=[ag_i[:]], outs=[ag_o[:]])

                # ---------------- conv2 A-table build (no transposes) ----------
                for ct in range(NC):
                    for w in range(SLOC):
                        win = ct * SLOC + w
                        lhs = xp.tile([COUT, 128], f32, tag="lhs2")
                        nc.sync.dma_start(lhs[:], ag_o[ct, :, w * 128:(w + 1) * 128])
                        pm = xp.tile([3, 128], f32, tag="posm")
                        nc.sync.dma_start(pm[:], posw_in[:, win * 128:(win + 1) * 128])
                        pb = pq.tile([128, COUT], f32, tag="pout")
                        nc.tensor.matmul(out=pb[:], lhsT=lhs[:], rhs=W2as[:], start=True, stop=False)
                        nc.tensor.matmul(out=pb[:], lhsT=pm[:], rhs=W2ps[:], start=False, stop=False)
                        nc.tensor.matmul(out=pb[:], lhsT=ones1[:], rhs=brow["b2"][:], start=False, stop=True)
                        oc = cp.tile([128, COUT], f32, tag="oc")
                        nc.vector.tensor_copy(out=oc[:], in_=pb[:])
                        mlim = min(128, _ceil(NLOC - w, SLOC))
                        base = 1 + ct * NLOC + w
                        m0 = max(0, min(mlim, _ceil(LO_LIM - base, SLOC)))
                        if m0 > 0:
                            d = table_lo[base:base + m0 * SLOC, :].rearrange(
                                "(m s) f -> m s f", s=SLOC)[:, 0, :]
                            nc.sync.dma_start(d, oc[0:m0, :])
                        if m0 < mlim:
                            b2 = base + m0 * SLOC - LO_LIM
                            d = table_hi[b2:b2 + (mlim - m0) * SLOC, :].rearrange(
                                "(m s) f -> m s f", s=SLOC)[:, 0, :]
                            nc.sync.dma_start(d, oc[m0:mlim, :])

                agg2 = gather_conv(2)

                # v2 = (agg2 - c2) * mask
                v2 = ap_.tile([128, SLOC, COUT], f32, tag="acc_hi")
                for s in range(SLOC):
                    pm = xp.tile([3, 128], f32, tag="posm")
                    nc.sync.dma_start(pm[:], posm_in[:, s * 128:(s + 1) * 128])
                    pc = pq.tile([128, COUT], f32, tag="pout")
                    nc.tensor.matmul(out=pc[:], lhsT=pm[:], rhs=W2ps[:], start=True, stop=True)
                    t_ = cp.tile([128, COUT], f32, tag="tv")
                    nc.vector.tensor_tensor(out=t_[:], in0=agg2[:, s, :], in1=pc[:], op=OP.subtract)
                    nc.vector.tensor_tensor(out=v2[:, s, :], in0=t_[:], in1=mask[:, s, :], op=OP.mult)

                arst2 = sing.tile([COUT, 2], f32, tag="arst2")
                stats_into(v2, arst2[:])
                nc.sync.dma_start(ar2_i[:], arst2[:])
                if timeline:
                    _t = sm.tile([COUT, 2], f32, tag="cc2")
                    nc.sync.dma_start(_t[:], ar2_i[:])
                    nc.sync.dma_start(ar2_o[:], _t[:])
                else:
                    nc.gpsimd.collective_compute("AllReduce", OP.add, replica_groups=groups,
                                                 ins=[ar2_i[:]], outs=[ar2_o[:]])
                arres2 = sing.tile([COUT, 2], f32, tag="arres2")
                nc.sync.dma_start(arres2[:], ar2_o[:])
                rows2 = bn_params(arres2[:, 0:1], arres2[:, 1:2], pvec["g2"], pvec["be2"], "bn2")

                # final = relu(bn2(v2) + bnl(skip))
                skg = st.tile([128, SLOC, COUT], f32, tag="stage")
                gather_chunked(skg[:], skipb[:], midx["sk"], 0, SLOC)
                fin = ap_.tile([128, SLOC, COUT], f32, tag="acc_hi2")
                nc.vector.tensor_tensor(out=fin[:], in0=v2[:],
                                        in1=rows2[:, 0:1, :].to_broadcast([128, SLOC, COUT]), op=OP.mult)
                nc.vector.tensor_tensor(out=fin[:], in0=fin[:],
                                        in1=rows2[:, 1:2, :].to_broadcast([128, SLOC, COUT]), op=OP.add)
                skbn = st.tile([128, SLOC, COUT], f32, tag="stage")
                nc.vector.tensor_tensor(out=skbn[:], in0=skg[:],
                                        in1=rowsl[:, 0:1, :].to_broadcast([128, SLOC, COUT]), op=OP.mult)
                nc.vector.tensor_tensor(out=skbn[:], in0=skbn[:],
                                        in1=rowsl[:, 1:2, :].to_broadcast([128, SLOC, COUT]), op=OP.add)
                nc.vector.tensor_tensor(out=fin[:], in0=fin[:], in1=skbn[:], op=OP.add)
                nc.vector.tensor_scalar(out=fin[:], in0=fin[:], scalar1=0.0, scalar2=None, op0=OP.max)
                nc.sync.dma_start(out_t[:].rearrange("(p s) f -> p s f", p=128), fin[:])

    nc.compile()
    return nc


def make_in_maps(inputs, cfg, per_core, posw):
    posw, posT = posw
    N, NC, CIN = cfg["N"], cfg["NC"], cfg["CIN"]
    NLOC = N // NC
    shared = dict(
        posT=posT,
        x=np.ascontiguousarray(np.asarray(inputs["x"], np.float32)),
        pos=np.ascontiguousarray(np.asarray(inputs["pos"], np.float32)),
        posw=posw,
        W1=np.asarray(inputs["W1"], np.float32),
        b1=np.asarray(inputs["b1"], np.float32).reshape(1, -1),
        W2=np.asarray(inputs["W2"], np.float32),
        b2=np.asarray(inputs["b2"], np.float32).reshape(1, -1),
        Wl=np.asarray(inputs["Wl"], np.float32),
        bl=np.asarray(inputs["bl"], np.float32).reshape(1, -1),
        g1=np.asarray(inputs["g1"], np.float32).reshape(-1, 1),
        be1=np.asarray(inputs["be1"], np.float32).reshape(-1, 1),
        g2=np.asarray(inputs["g2"], np.float32).reshape(-1, 1),
        be2=np.asarray(inputs["be2"], np.float32).reshape(-1, 1),
        gl=np.asarray(inputs["gl"], np.float32).reshape(-1, 1),
        bel=np.asarray(inputs["bel"], np.float32).reshape(-1, 1),
    )
    in_maps = []
    for c in range(NC):
        m = dict(shared)
        m["xs"] = np.ascontiguousarray(shared["x"][c * NLOC:(c + 1) * NLOC])
        pc = per_core[c]
        m["gi_lo"] = pc["gi_lo"]
        m["gi_hi"] = pc["gi_hi"]
        m["mg_lo"] = pc["mg_lo"]
        m["mg_hi"] = pc["mg_hi"]
        m["mg_skip"] = pc["mg_skip"]
        m["posm"] = pc["posm"]
        in_maps.append(m)
    return in_maps


_CACHE = {}


def run(inputs, cfg, use_sim=False, trace=False):
    per_core, posw, sched = host_prep(inputs["edge_index"], inputs["pos"], cfg)
    key = (cfg["N"], tuple(sched[0]), tuple(sched[1]))
    if key not in _CACHE:
        _CACHE[key] = build_bass(cfg, sched)
    nc = _CACHE[key]
    in_maps = make_in_maps(inputs, cfg, per_core, posw)
    NC = cfg["NC"]
    NLOC = cfg["N"] // NC
    if use_sim:
        from concourse.bass_interp import MultiCoreSim
        sim = MultiCoreSim(nc, num_cores=NC, require_finite=False, require_nnan=False)
        for c in range(NC):
            for k, v in in_maps[c].items():
                sim.cores[c].tensor(k)[:] = v
        sim.simulate(check_with_hw=False)
        outs = [np.array(sim.cores[c].tensor("out")) for c in range(NC)]
        res = None
    else:
        from concourse.bass_utils import run_bass_kernel_spmd
        res = run_bass_kernel_spmd(nc, in_maps, core_ids=list(range(NC)), trace=trace)
        outs = [res.results[c]["out"] for c in range(NC)]
    full = np.concatenate([o[:NLOC] for o in outs], axis=0)
    return full, res


def kernel(**inputs):
    out, _ = run(inputs, FULL_CFG, use_sim=False)
    return out



# revision 6
# speedup vs baseline: 422.2472x; 422.2472x over previous
"""Trainium2 Bass kernel for nn_BlockConv (PointNet-style GNN block), 8 cores.

Algebraic core: msg_e = concat(x_src, pos_src-pos_dst) @ W + b
  = A[src] - C[dst], with A = concat(x,pos)@W + b (per-node table) and
  C = pos@W[-3:] (per-dst, constant within a segment). segment_max over
  dst = (gather+max of A rows) - C[dst]. Pure memory problem.

Distribution: dst-sharded; each core computes the A-table rows for ITS
nodes only (slot layout) and one AllGather per conv materializes the
full gather table in shared HBM (ag_o = [NC, NSLOT+128, COUT]; rows
NSLOT..NSLOT+127 of every slice are -BIG dummies for pass padding).
Global row of node n = (n//NLOC)*(NSLOT+128) + n%NLOC. Skip path is
computed in slot layout from a strided x read (no regather); BN stats
via a tiny AllReduce of per-core partial sums (valid-partition masked).

Gather: dma_gather (int16 idx) over lo/hi row windows split at 32768;
per window the core's nodes are degree-sorted so pass k covers a slot
prefix; DVE max chains accumulate; one HBM round-trip re-gathers both
accumulators in node order and maxes them.
"""
import sys
import numpy as np

if "/opt/trn_rl_repo" not in sys.path:
    sys.path.insert(0, "/opt/trn_rl_repo")

BIG_NEG = -1.0e30
EPS = 1e-5

FULL_CFG = dict(N=50000, E=800000, CIN=64, COUT=128, NC=8, LO_LIM=32768)
MINI_CFG = dict(N=2048, E=16384, CIN=64, COUT=128, NC=8, LO_LIM=1024)
MID_CFG = dict(N=16384, E=262144, CIN=64, COUT=128, NC=8, LO_LIM=8192)


def _ceil(a, b):
    return (a + b - 1) // b


def _derived(cfg):
    N, NC = cfg["N"], cfg["NC"]
    NLOC = N // NC
    SLOC = _ceil(NLOC, 128)       # smallest divisor of NLOC >= ceil(NLOC/128)
    while NLOC % SLOC:
        SLOC += 1
    NSLOT = SLOC * 128
    NSLOTP = NSLOT + 128          # +128 dummy -BIG rows per core slice
    RTOT = NC * NSLOTP
    PV = min(128, NLOC // SLOC)   # valid partitions (slots p*SLOC+s < NLOC)
    return NLOC, SLOC, NSLOT, NSLOTP, RTOT, PV


def _wrap16(ids):
    """flat int list (len % 128 == 0) -> [128, len//16] int16 wrapped:
    unwrapped[j] = g[j%16, j//16], replicated over the 8 core groups."""
    a = np.asarray(ids, np.int64)
    assert a.size % 128 == 0 and a.min() >= 0 and a.max() < 32768
    g = a.reshape(a.size // 16, 16).T.astype(np.int16)   # [16, L/16]
    return np.tile(g, (8, 1))                            # [128, L/16]


def host_prep(edge_index, pos, cfg):
    N, NC, LO_LIM = cfg["N"], cfg["NC"], cfg["LO_LIM"]
    NLOC, SLOC, NSLOT, NSLOTP, RTOT, PV = _derived(cfg)
    src = np.asarray(edge_index[0], np.int64)
    dst = np.asarray(edge_index[1], np.int64)
    rows = (src // NLOC) * NSLOTP + (src % NLOC)   # global table row of src
    core_of = dst // NLOC
    dummy_lo = NSLOT                                # core 0 dummy block
    dummy_hi = (NC - 1) * NSLOTP + NSLOT - LO_LIM   # last core dummy block

    sides = [[], []]     # sides[0][c] = lo side of core c
    for c in range(NC):
        m = core_of == c
        s_rows = rows[m]
        d_loc = dst[m] - c * NLOC
        for si, sel in ((0, s_rows < LO_LIM), (1, s_rows >= LO_LIM)):
            s = s_rows[sel] - (0 if si == 0 else LO_LIM)
            d = d_loc[sel]
            deg = np.bincount(d, minlength=NSLOT)
            order = np.argsort(-deg, kind="stable")
            slot_of = np.empty(NSLOT, np.int64)
            slot_of[order] = np.arange(NSLOT)
            isort = np.argsort(d, kind="stable")
            starts = np.zeros(NSLOT + 1, np.int64)
            np.cumsum(deg, out=starts[1:])
            sides[si].append({"deg": deg, "order": order, "slot_of": slot_of,
                              "s_sorted": s[isort], "starts": starts,
                              "cnts": np.sort(deg)[::-1]})

    sched = []
    for si in range(2):
        Sk = []
        kmax = max(int(sd["cnts"][0]) for sd in sides[si])
        for k in range(kmax):
            cnt = max(int((sd["cnts"] > k).sum()) for sd in sides[si])
            if cnt == 0:
                break
            Sk.append(_ceil(cnt, 128))
        sched.append(Sk)

    j = np.arange(NSLOT)
    n_of_j = (j % 128) * SLOC + j // 128

    import ml_dtypes
    per_core = []
    for c in range(NC):
        blocks = {0: [], 1: []}
        for si in range(2):
            sd = sides[si][c]
            dummy = dummy_lo if si == 0 else dummy_hi
            for k, S in enumerate(sched[si]):
                L = S * 128
                ids = np.full(L, dummy, np.int64)
                nsl = int((sd["cnts"] > k).sum())
                nodes = sd["order"][:nsl]
                ids[:nsl] = sd["s_sorted"][sd["starts"][nodes] + k]
                blocks[si].append(_wrap16(ids))
        gi_lo = (np.concatenate(blocks[0], axis=1) if blocks[0]
                 else np.zeros((128, 8), np.int16))
        gi_hi = (np.concatenate(blocks[1], axis=1) if blocks[1]
                 else np.zeros((128, 8), np.int16))
        mg_lo = _wrap16(sides[0][c]["slot_of"][n_of_j])
        mg_hi = _wrap16(sides[1][c]["slot_of"][n_of_j] + NSLOT)
        gnode = np.minimum(c * NLOC + n_of_j, N - 1)
        posm = np.ascontiguousarray(
            np.asarray(pos)[gnode].T).astype(ml_dtypes.bfloat16)
        per_core.append({"gi_lo": gi_lo, "gi_hi": gi_hi, "mg_lo": mg_lo,
                         "mg_hi": mg_hi, "posm": posm})
    return per_core, sched


def build_bass(cfg, sched, reps=1, timeline=False):
    import concourse.bass as bass
    import concourse.bacc as bacc
    import concourse.tile as tile
    from concourse import mybir
    from concourse.masks import make_identity
    import contextlib

    N, NC = cfg["N"], cfg["NC"]
    CIN, COUT = cfg["CIN"], cfg["COUT"]
    LO_LIM = cfg["LO_LIM"]
    NLOC, SLOC, NSLOT, NSLOTP, RTOT, PV = _derived(cfg)
    HI_R = RTOT - LO_LIM
    f32, bf16, i16 = mybir.dt.float32, mybir.dt.bfloat16, mybir.dt.int16
    OP = mybir.AluOpType
    AX = mybir.AxisListType
    AF = mybir.ActivationFunctionType

    nc = bacc.Bacc(num_devices=(1 if timeline else NC), name="blockconv")

    xs_in = nc.dram_tensor("xs", [NLOC, CIN], f32, kind="ExternalInput")
    posm_in = nc.dram_tensor("posm", [3, NSLOT], bf16, kind="ExternalInput")
    wt = {}
    for nm, shp in (("W1", [CIN + 3, COUT]), ("b1", [1, COUT]),
                    ("W2", [COUT + 3, COUT]), ("b2", [1, COUT]),
                    ("Wl", [CIN, COUT]), ("bl", [1, COUT]),
                    ("g1", [COUT, 1]), ("be1", [COUT, 1]), ("g2", [COUT, 1]),
                    ("be2", [COUT, 1]), ("gl", [COUT, 1]), ("bel", [COUT, 1])):
        wt[nm] = nc.dram_tensor(nm, shp, f32, kind="ExternalInput")

    Wlo = max(sum(sched[0]), 1) * 8
    Whi = max(sum(sched[1]), 1) * 8
    gi_lo_in = nc.dram_tensor("gi_lo", [128, Wlo], i16, kind="ExternalInput")
    gi_hi_in = nc.dram_tensor("gi_hi", [128, Whi], i16, kind="ExternalInput")
    mg_lo_in = nc.dram_tensor("mg_lo", [128, NSLOT // 16], i16, kind="ExternalInput")
    mg_hi_in = nc.dram_tensor("mg_hi", [128, NSLOT // 16], i16, kind="ExternalInput")

    out_t = nc.dram_tensor("out", [NSLOT, COUT], f32, kind="ExternalOutput")

    shared = "Local" if timeline else "Shared"
    ag1_i = nc.dram_tensor("ag1_in", [NSLOTP, COUT], f32)
    ag1_o = nc.dram_tensor("ag1_out", [NC, NSLOTP, COUT], f32, addr_space=shared)
    ag2_i = nc.dram_tensor("ag2_in", [NSLOTP, COUT], f32)
    ag2_o = nc.dram_tensor("ag2_out", [NC, NSLOTP, COUT], f32, addr_space=shared)
    mbuf = nc.dram_tensor("mbuf", [2 * NSLOT, COUT], f32)
    skipb = nc.dram_tensor("skipbuf", [NLOC, COUT], f32)
    ar_i = nc.dram_tensor("ar_in", [COUT, 4], f32)
    ar_o = nc.dram_tensor("ar_out", [COUT, 4], f32, addr_space=shared)
    rowbuf = nc.dram_tensor("rowbuf", [6, COUT], f32)
    ar2_i = nc.dram_tensor("ar2_in", [COUT, 2], f32)
    ar2_o = nc.dram_tensor("ar2_out", [COUT, 2], f32, addr_space=shared)
    groups = [list(range(NC))]

    with tile.TileContext(nc) as tc:
        ctx = contextlib.ExitStack()
        with ctx:
            sing = ctx.enter_context(tc.tile_pool(name="sing", bufs=1))
            xp = ctx.enter_context(tc.tile_pool(name="xp", bufs=3))
            pp = ctx.enter_context(tc.tile_pool(name="pp", bufs=2, space="PSUM"))
            pq = ctx.enter_context(tc.tile_pool(name="pq", bufs=2, space="PSUM"))
            pr = ctx.enter_context(tc.tile_pool(name="pr", bufs=1, space="PSUM"))
            cp = ctx.enter_context(tc.tile_pool(name="cp", bufs=4))
            ap_ = ctx.enter_context(tc.tile_pool(name="ap", bufs=1))
            st = ctx.enter_context(tc.tile_pool(name="st", bufs=2))
            sm = ctx.enter_context(tc.tile_pool(name="sm", bufs=2))

            ident = sing.tile([128, 128], f32)
            make_identity(nc, ident)
            ones1 = sing.tile([1, 128], f32)
            nc.vector.memset(ones1[:], 1.0)
            onesp = sing.tile([128, 1], f32)
            nc.vector.memset(onesp[:], 1.0)
            onespv = sing.tile([128, 1], f32)
            nc.vector.memset(onespv[:], 0.0)
            nc.vector.memset(onespv[0:PV], 1.0)
            negbig = sing.tile([128, COUT], f32)
            nc.vector.memset(negbig[:], BIG_NEG)
            epsv = sing.tile([COUT, 1], f32)
            nc.vector.memset(epsv[:], EPS)

            W1s = sing.tile([CIN + 3, COUT], f32)
            nc.sync.dma_start(W1s[:], wt["W1"][:])
            W1pb = sing.tile([3, COUT], bf16)
            nc.vector.tensor_copy(out=W1pb[:], in_=W1s[CIN:CIN + 3, :])
            W2as = sing.tile([COUT, COUT], f32)
            nc.sync.dma_start(W2as[:], wt["W2"][0:COUT, :])
            W2pf = sing.tile([3, COUT], f32)
            nc.sync.dma_start(W2pf[:], wt["W2"][COUT:COUT + 3, :])
            W2pb = sing.tile([3, COUT], bf16)
            nc.vector.tensor_copy(out=W2pb[:], in_=W2pf[:])
            Wls = sing.tile([CIN, COUT], f32)
            nc.sync.dma_start(Wls[:], wt["Wl"][:])
            brow = {}
            for nm in ("b1", "b2", "bl"):
                t = sing.tile([1, COUT], f32, tag=f"br_{nm}")
                nc.sync.dma_start(t[:], wt[nm][:])
                brow[nm] = t
            pvec = {}
            for nm in ("g1", "be1", "g2", "be2", "gl", "bel"):
                v = sing.tile([COUT, 1], f32, tag=f"pv_{nm}")
                nc.sync.dma_start(v[:], wt[nm][:])
                pvec[nm] = v

            posmb = sing.tile([3, NSLOT], bf16)
            nc.sync.dma_start(posmb[:], posm_in[:])
            idx_lo = sing.tile([128, Wlo], i16)
            nc.sync.dma_start(idx_lo[:], gi_lo_in[:])
            idx_hi = sing.tile([128, Whi], i16)
            nc.sync.dma_start(idx_hi[:], gi_hi_in[:])
            midx = {}
            for nm, t_ in (("lo", mg_lo_in), ("hi", mg_hi_in)):
                m_ = sing.tile([128, NSLOT // 16], i16, tag=f"mi_{nm}")
                nc.sync.dma_start(m_[:], t_[:])
                midx[nm] = m_

            for _rep in range(reps):
                # -BIG dummy rows NSLOT..NSLOT+127 of this core's slices
                nc.sync.dma_start(ag1_i[NSLOT:NSLOTP, :], negbig[:])
                nc.sync.dma_start(ag2_i[NSLOT:NSLOTP, :], negbig[:])

                # ---------- build a1 + skip in slot layout -------------
                xbig = sing.tile([128, SLOC, CIN], f32, tag="xbig")
                nc.vector.memset(xbig[:], 0.0)
                nc.sync.dma_start(
                    xbig[0:PV], xs_in[:].rearrange("(p s) c -> p s c", s=SLOC))
                a1 = ap_.tile([128, SLOC, COUT], f32, tag="big1")
                skt = ap_.tile([128, SLOC, COUT], f32, tag="big2")
                for s in range(SLOC):
                    ps = pp.tile([128, 128], f32, tag="pst")
                    nc.tensor.transpose(out=ps[0:CIN, :], in_=xbig[:, s, :], identity=ident[:])
                    xT = xp.tile([CIN, 128], f32, tag="xT")
                    nc.scalar.copy(out=xT[:], in_=ps[0:CIN, :])
                    pb = pq.tile([128, COUT], f32, tag="pout")
                    nc.tensor.matmul(out=pb[:], lhsT=xT[:], rhs=W1s[0:CIN, :], start=True, stop=False)
                    nc.tensor.matmul(out=pb[:], lhsT=posmb[:, s * 128:(s + 1) * 128], rhs=W1pb[:], start=False, stop=False)
                    nc.tensor.matmul(out=pb[:], lhsT=ones1[:], rhs=brow["b1"][:], start=False, stop=True)
                    nc.vector.tensor_copy(out=a1[:, s, :], in_=pb[:])
                    pl = pq.tile([128, COUT], f32, tag="pout")
                    nc.tensor.matmul(out=pl[:], lhsT=xT[:], rhs=Wls[:], start=True, stop=False)
                    nc.tensor.matmul(out=pl[:], lhsT=ones1[:], rhs=brow["bl"][:], start=False, stop=True)
                    nc.scalar.copy(out=skt[:, s, :], in_=pl[:])

                # skip stats (valid partitions only; all s cover NLOC rows)
                arst = sing.tile([COUT, 4], f32)

                def stats_into(vtile, arcols, ones_vec):
                    red = sm.tile([128, COUT], f32, tag="red")
                    nc.vector.tensor_reduce(out=red[:], in_=vtile[:].rearrange("p s f -> p f s"),
                                            op=OP.add, axis=AX.X)
                    vsq = st.tile([128, SLOC, COUT], f32, tag="stage")
                    nc.vector.tensor_tensor(out=vsq[:], in0=vtile[:], in1=vtile[:], op=OP.mult)
                    redq = sm.tile([128, COUT], f32, tag="redq")
                    nc.vector.tensor_reduce(out=redq[:], in_=vsq[:].rearrange("p s f -> p f s"),
                                            op=OP.add, axis=AX.X)
                    pst_ = pr.tile([COUT, 2], f32, tag="pstat")
                    nc.tensor.matmul(out=pst_[:, 0:1], lhsT=red[:], rhs=ones_vec[:], start=True, stop=True)
                    nc.tensor.matmul(out=pst_[:, 1:2], lhsT=redq[:], rhs=ones_vec[:], start=True, stop=True)
                    nc.vector.tensor_copy(out=arcols, in_=pst_[:])

                stats_into(skt, arst[:, 2:4], onespv)
                # spill skip to HBM (node order rows 0..NLOC)
                nc.sync.dma_start(
                    skipb[:].rearrange("(p s) f -> p s f", s=SLOC), skt[0:PV])
                # a1 -> ag1_i rows 0..NLOC
                nc.sync.dma_start(
                    ag1_i[0:NLOC, :].rearrange("(p s) f -> p s f", s=SLOC), a1[0:PV])

                def allgather(src, dst):
                    if timeline:
                        for q in range(_ceil(NSLOTP, 128)):
                            r0, r1 = q * 128, min((q + 1) * 128, NSLOTP)
                            t_ = cp.tile([128, COUT], f32, tag="agb")
                            nc.sync.dma_start(t_[0:r1 - r0], src[r0:r1, :])
                            nc.sync.dma_start(dst[0, r0:r1, :], t_[0:r1 - r0])
                    else:
                        nc.gpsimd.collective_compute(
                            "AllGather", OP.bypass, replica_groups=groups,
                            ins=[src[:]], outs=[dst[:]])

                allgather(ag1_i, ag1_o)

                # ---------------- gather-max passes ----------------
                GMAX = 8   # max 8*128=1024 indices per dma_gather (SWDGE ring cap)

                def gather_chunked(dst3, in_ap, idxt, chunk0, nchunks):
                    a = 0
                    while a < nchunks:
                        b = min(a + GMAX, nchunks)
                        nc.gpsimd.dma_gather(
                            out_ap=dst3[:, a:b, :], in_ap=in_ap,
                            idxs_ap=idxt[:, (chunk0 + a) * 8:(chunk0 + b) * 8],
                            num_idxs=(b - a) * 128, num_idxs_reg=(b - a) * 128,
                            elem_size=COUT)
                        a = b

                def gather_conv(ag_o, acc_tags):
                    flat = ag_o[:].rearrange("c n f -> (c n) f")
                    acc = {}
                    for snm, tg in zip(("lo", "hi"), acc_tags):
                        a = ap_.tile([128, SLOC, COUT], f32, tag=tg)
                        nc.gpsimd.memset(a[:], BIG_NEG)
                        acc[snm] = a
                    for snm, idxt, w0, w1 in (("lo", idx_lo, 0, LO_LIM),
                                              ("hi", idx_hi, LO_LIM, RTOT)):
                        off = 0
                        for k, S in enumerate(sched[0 if snm == "lo" else 1]):
                            stg = st.tile([128, SLOC, COUT], f32, tag="stage")
                            gather_chunked(stg[:, 0:S, :], flat[w0:w1, :],
                                           idxt, off // 8, S)
                            nc.vector.tensor_tensor(
                                out=acc[snm][:, 0:S, :], in0=acc[snm][:, 0:S, :],
                                in1=stg[:, 0:S, :], op=OP.max)
                            off += 8 * S
                    # merge via HBM round-trip, node order
                    nc.sync.dma_start(
                        mbuf[0:NSLOT, :].rearrange("(s p) f -> p s f", p=128), acc["lo"][:])
                    nc.sync.dma_start(
                        mbuf[NSLOT:2 * NSLOT, :].rearrange("(s p) f -> p s f", p=128), acc["hi"][:])
                    g1t = st.tile([128, SLOC, COUT], f32, tag="stage")
                    gather_chunked(g1t[:], mbuf[:], midx["lo"], 0, SLOC)
                    g2t = st.tile([128, SLOC, COUT], f32, tag="stage")
                    gather_chunked(g2t[:], mbuf[:], midx["hi"], 0, SLOC)
                    agg = ap_.tile([128, SLOC, COUT], f32, tag=acc_tags[0])
                    nc.vector.tensor_tensor(out=agg[:], in0=g1t[:], in1=g2t[:], op=OP.max)
                    return agg

                agg1 = gather_conv(ag1_o, ("big1", "big2"))

                # v1 = (agg1 - pc1) * (agg1 > -1e29), per chunk s
                def v_compute(agg, Wp, vtag):
                    v = ap_.tile([128, SLOC, COUT], f32, tag=vtag)
                    for s in range(SLOC):
                        pc = pq.tile([128, COUT], f32, tag="pout")
                        nc.tensor.matmul(out=pc[:], lhsT=posmb[:, s * 128:(s + 1) * 128],
                                         rhs=Wp[:], start=True, stop=True)
                        msk = cp.tile([128, COUT], f32, tag="msk")
                        nc.vector.tensor_scalar(out=msk[:], in0=agg[:, s, :], scalar1=-1.0e29,
                                                scalar2=None, op0=OP.is_gt)
                        t_ = cp.tile([128, COUT], f32, tag="tv")
                        nc.vector.tensor_tensor(out=t_[:], in0=agg[:, s, :], in1=pc[:], op=OP.subtract)
                        nc.vector.tensor_tensor(out=v[:, s, :], in0=t_[:], in1=msk[:], op=OP.mult)
                    return v

                v1 = v_compute(agg1, W1pb, "big2")
                stats_into(v1, arst[:, 0:2], onesp)
                nc.sync.dma_start(ar_i[:], arst[:])
                if timeline:
                    _t = sm.tile([COUT, 4], f32, tag="cc1")
                    nc.sync.dma_start(_t[:], ar_i[:])
                    nc.sync.dma_start(ar_o[:], _t[:])
                else:
                    nc.gpsimd.collective_compute("AllReduce", OP.add, replica_groups=groups,
                                                 ins=[ar_i[:]], outs=[ar_o[:]])
                arres = sing.tile([COUT, 4], f32, tag="arres")
                nc.sync.dma_start(arres[:], ar_o[:])

                # BN params: scale = g * rsqrt(var+eps), shift = be - mean*scale
                def bn_params(sum_ap, sq_ap, g_v, be_v, tagp):
                    mean = sm.tile([COUT, 1], f32, tag=f"{tagp}_m")
                    nc.vector.tensor_scalar(out=mean[:], in0=sum_ap, scalar1=1.0 / N,
                                            scalar2=None, op0=OP.mult)
                    ex2 = sm.tile([COUT, 1], f32, tag=f"{tagp}_e")
                    nc.vector.tensor_scalar(out=ex2[:], in0=sq_ap, scalar1=1.0 / N,
                                            scalar2=None, op0=OP.mult)
                    m2 = sm.tile([COUT, 1], f32, tag=f"{tagp}_m2")
                    nc.vector.tensor_tensor(out=m2[:], in0=mean[:], in1=mean[:], op=OP.mult)
                    var = sm.tile([COUT, 1], f32, tag=f"{tagp}_v")
                    nc.vector.tensor_tensor(out=var[:], in0=ex2[:], in1=m2[:], op=OP.subtract)
                    sd = sm.tile([COUT, 1], f32, tag=f"{tagp}_sd")
                    nc.scalar.activation(out=sd[:], in_=var[:], func=AF.Sqrt, bias=epsv[:], scale=1.0)
                    rstd = sm.tile([COUT, 1], f32, tag=f"{tagp}_r")
                    nc.vector.reciprocal(out=rstd[:], in_=sd[:])
                    ssh = sm.tile([COUT, 2], f32, tag=f"{tagp}_ssh")
                    nc.vector.tensor_tensor(out=ssh[:, 0:1], in0=rstd[:], in1=g_v[:], op=OP.mult)
                    ms = sm.tile([COUT, 1], f32, tag=f"{tagp}_ms")
                    nc.vector.tensor_tensor(out=ms[:], in0=mean[:], in1=ssh[:, 0:1], op=OP.mult)
                    nc.vector.tensor_tensor(out=ssh[:, 1:2], in0=be_v[:], in1=ms[:], op=OP.subtract)
                    # transpose [COUT,2] -> [2, COUT] rows (scale row 0, shift row 1)
                    prow = pr.tile([2, COUT], f32, tag="prow")
                    nc.tensor.transpose(out=prow[:], in_=ssh[:], identity=ident[:])
                    rows = sing.tile([2, COUT], f32, tag=f"{tagp}_rows")
                    nc.vector.tensor_copy(out=rows[:], in_=prow[:])
                    slot = {"bn1": 0, "bnl": 2, "bn2": 4}[tagp]
                    nc.sync.dma_start(rowbuf[slot:slot + 2, :], rows[:])
                    bc = sing.tile([128, 2, COUT], f32, tag=f"{tagp}_bc")
                    rap = rowbuf[slot:slot + 2, :]
                    nc.sync.dma_start(bc[:], bass.AP(tensor=rap.tensor, offset=rap.offset,
                                                     ap=[[0, 128]] + list(rap.ap)))
                    return bc

                rows1 = bn_params(arres[:, 0:1], arres[:, 1:2], pvec["g1"], pvec["be1"], "bn1")
                rowsl = bn_params(arres[:, 2:3], arres[:, 3:4], pvec["gl"], pvec["bel"], "bnl")

                # h1 = relu(v1*scale1 + shift1)
                h1 = ap_.tile([128, SLOC, COUT], f32, tag="big3")
                sc3 = rows1[:, 0:1, :].to_broadcast([128, SLOC, COUT])
                sh3 = rows1[:, 1:2, :].to_broadcast([128, SLOC, COUT])
                nc.vector.tensor_tensor(out=h1[:], in0=v1[:], in1=sc3, op=OP.mult)
                nc.vector.tensor_tensor(out=h1[:], in0=h1[:], in1=sh3, op=OP.add)
                nc.vector.tensor_scalar(out=h1[:], in0=h1[:], scalar1=0.0, scalar2=None, op0=OP.max)

                # ---------- a2 = [h1|pos]@W2 + b2 (slot layout) --------
                a2 = ap_.tile([128, SLOC, COUT], f32, tag="big1")
                for s in range(SLOC):
                    ph = pp.tile([128, 128], f32, tag="pst")
                    nc.tensor.transpose(out=ph[:], in_=h1[:, s, :], identity=ident[:])
                    hT = xp.tile([128, 128], f32, tag="hT")
                    nc.scalar.copy(out=hT[:], in_=ph[:])
                    pb = pq.tile([128, COUT], f32, tag="pout")
                    nc.tensor.matmul(out=pb[:], lhsT=hT[:], rhs=W2as[:], start=True, stop=False)
                    nc.tensor.matmul(out=pb[:], lhsT=posmb[:, s * 128:(s + 1) * 128], rhs=W2pb[:], start=False, stop=False)
                    nc.tensor.matmul(out=pb[:], lhsT=ones1[:], rhs=brow["b2"][:], start=False, stop=True)
                    nc.vector.tensor_copy(out=a2[:, s, :], in_=pb[:])
                nc.sync.dma_start(
                    ag2_i[0:NLOC, :].rearrange("(p s) f -> p s f", s=SLOC), a2[0:PV])
                allgather(ag2_i, ag2_o)

                agg2 = gather_conv(ag2_o, ("big1", "big2"))
                v2 = v_compute(agg2, W2pb, "big2")

                arst2 = sing.tile([COUT, 2], f32, tag="arst2")
                stats_into(v2, arst2[:], onesp)
                nc.sync.dma_start(ar2_i[:], arst2[:])
                if timeline:
                    _t = sm.tile([COUT, 2], f32, tag="cc2")
                    nc.sync.dma_start(_t[:], ar2_i[:])
                    nc.sync.dma_start(ar2_o[:], _t[:])
                else:
                    nc.gpsimd.collective_compute("AllReduce", OP.add, replica_groups=groups,
                                                 ins=[ar2_i[:]], outs=[ar2_o[:]])
                arres2 = sing.tile([COUT, 2], f32, tag="arres2")
                nc.sync.dma_start(arres2[:], ar2_o[:])
                rows2 = bn_params(arres2[:, 0:1], arres2[:, 1:2], pvec["g2"], pvec["be2"], "bn2")

                # final = relu(bn2(v2) + bnl(skip))
                skg = st.tile([128, SLOC, COUT], f32, tag="stage")
                nc.vector.memset(skg[:], 0.0)
                nc.sync.dma_start(
                    skg[0:PV], skipb[:].rearrange("(p s) f -> p s f", s=SLOC))
                fin = ap_.tile([128, SLOC, COUT], f32, tag="big3")
                nc.vector.tensor_tensor(out=fin[:], in0=v2[:],
                                        in1=rows2[:, 0:1, :].to_broadcast([128, SLOC, COUT]), op=OP.mult)
                nc.vector.tensor_tensor(out=fin[:], in0=fin[:],
                                        in1=rows2[:, 1:2, :].to_broadcast([128, SLOC, COUT]), op=OP.add)
                skbn = st.tile([128, SLOC, COUT], f32, tag="stage")
                nc.vector.tensor_tensor(out=skbn[:], in0=skg[:],
                                        in1=rowsl[:, 0:1, :].to_broadcast([128, SLOC, COUT]), op=OP.mult)
                nc.vector.tensor_tensor(out=skbn[:], in0=skbn[:],
                                        in1=rowsl[:, 1:2, :].to_broadcast([128, SLOC, COUT]), op=OP.add)
                nc.vector.tensor_tensor(out=fin[:], in0=fin[:], in1=skbn[:], op=OP.add)
                nc.vector.tensor_scalar(out=fin[:], in0=fin[:], scalar1=0.0, scalar2=None, op0=OP.max)
                nc.sync.dma_start(out_t[:].rearrange("(p s) f -> p s f", p=128), fin[:])

    nc.compile()
    return nc


def make_in_maps(inputs, cfg, per_core):
    N, NC, CIN = cfg["N"], cfg["NC"], cfg["CIN"]
    NLOC = N // NC
    x = np.ascontiguousarray(np.asarray(inputs["x"], np.float32))
    shared = dict(
        W1=np.asarray(inputs["W1"], np.float32),
        b1=np.asarray(inputs["b1"], np.float32).reshape(1, -1),
        W2=np.asarray(inputs["W2"], np.float32),
        b2=np.asarray(inputs["b2"], np.float32).reshape(1, -1),
        Wl=np.asarray(inputs["Wl"], np.float32),
        bl=np.asarray(inputs["bl"], np.float32).reshape(1, -1),
        g1=np.asarray(inputs["g1"], np.float32).reshape(-1, 1),
        be1=np.asarray(inputs["be1"], np.float32).reshape(-1, 1),
        g2=np.asarray(inputs["g2"], np.float32).reshape(-1, 1),
        be2=np.asarray(inputs["be2"], np.float32).reshape(-1, 1),
        gl=np.asarray(inputs["gl"], np.float32).reshape(-1, 1),
        bel=np.asarray(inputs["bel"], np.float32).reshape(-1, 1),
    )
    in_maps = []
    for c in range(NC):
        m = dict(shared)
        m["xs"] = np.ascontiguousarray(x[c * NLOC:(c + 1) * NLOC])
        pc = per_core[c]
        for k in ("gi_lo", "gi_hi", "mg_lo", "mg_hi", "posm"):
            m[k] = pc[k]
        in_maps.append(m)
    return in_maps


_CACHE = {}


def run(inputs, cfg, use_sim=False, trace=False):
    per_core, sched = host_prep(inputs["edge_index"], inputs["pos"], cfg)
    key = (cfg["N"], tuple(sched[0]), tuple(sched[1]))
    if key not in _CACHE:
        _CACHE[key] = build_bass(cfg, sched)
    nc = _CACHE[key]
    in_maps = make_in_maps(inputs, cfg, per_core)
    NC = cfg["NC"]
    NLOC = cfg["N"] // NC
    if use_sim:
        from concourse.bass_interp import MultiCoreSim
        sim = MultiCoreSim(nc, num_cores=NC, require_finite=False, require_nnan=False)
        for c in range(NC):
            for k, v in in_maps[c].items():
                sim.cores[c].tensor(k)[:] = v
        sim.simulate(check_with_hw=False)
        outs = [np.array(sim.cores[c].tensor("out")) for c in range(NC)]
        res = None
    else:
        from concourse.bass_utils import run_bass_kernel_spmd
        res = run_bass_kernel_spmd(nc, in_maps, core_ids=list(range(NC)), trace=trace)
        outs = [res.results[c]["out"] for c in range(NC)]
    full = np.concatenate([o[:NLOC] for o in outs], axis=0)
    return full, res


def kernel(**inputs):
    out, _ = run(inputs, FULL_CFG, use_sim=False)
    return out


# revision 13
# speedup vs baseline: 463.6059x; 1.0979x over previous
"""Trainium2 Bass kernel for nn_BlockConv (PointNet-style GNN block), 8 cores.

Algebraic core: msg_e = concat(x_src, pos_src-pos_dst) @ W + b
  = A[src] - C[dst], with A = concat(x,pos)@W + b (per-node table) and
  C = pos@W[-3:] (per-dst, constant within a segment). segment_max over
  dst = (gather+max of A rows) - C[dst]. Pure memory problem.

Distribution: dst-sharded; each core computes the A-table rows for ITS
nodes only (slot layout, fp16) and one AllGather per conv materializes
the full gather table in shared HBM (ag_o = [NC, NSLOT+128, COUT]; rows
NSLOT..NSLOT+127 of every slice are -BIG dummies for pass padding).
Global row of node n = (n//NLOC)*(NSLOT+128) + n%NLOC. Skip path is
computed in slot layout from a strided x read (no regather); BN stats
are chained PE matmuls (sum via ones vector, sum-of-squares via v^T v
diagonal), AllReduced across cores.

Gather: dma_gather (int16 idx) over lo/hi row windows split at 32768;
per window the core's nodes are degree-sorted so pass k covers a slot
prefix; DVE max chains accumulate; one HBM round-trip re-gathers both
accumulators in node order and maxes them.
"""
import sys
import numpy as np

if "/opt/trn_rl_repo" not in sys.path:
    sys.path.insert(0, "/opt/trn_rl_repo")

BIG_NEG = -1.0e30
BIG_NEG16 = -60000.0
EPS = 1e-5

FULL_CFG = dict(N=50000, E=800000, CIN=64, COUT=128, NC=8, LO_LIM=32768)
MINI_CFG = dict(N=2048, E=16384, CIN=64, COUT=128, NC=8, LO_LIM=1024)
MID_CFG = dict(N=16384, E=262144, CIN=64, COUT=128, NC=8, LO_LIM=8192)


def _ceil(a, b):
    return (a + b - 1) // b


def _derived(cfg):
    N, NC = cfg["N"], cfg["NC"]
    NLOC = N // NC
    SLOC = _ceil(NLOC, 128)       # smallest divisor of NLOC >= ceil(NLOC/128)
    while NLOC % SLOC:
        SLOC += 1
    NSLOT = SLOC * 128
    NSLOTP = NSLOT + 128          # +128 dummy -BIG rows per core slice
    RTOT = NC * NSLOTP
    PV = min(128, NLOC // SLOC)   # valid partitions (slots p*SLOC+s < NLOC)
    return NLOC, SLOC, NSLOT, NSLOTP, RTOT, PV


def _wrap16(ids):
    """flat int list (len % 128 == 0) -> [128, len//16] int16 wrapped:
    unwrapped[j] = g[j%16, j//16], replicated over the 8 core groups."""
    a = np.asarray(ids, np.int64)
    assert a.size % 128 == 0 and a.min() >= 0 and a.max() < 32768
    g = a.reshape(a.size // 16, 16).T.astype(np.int16)   # [16, L/16]
    return np.tile(g, (8, 1))                            # [128, L/16]


def host_prep(edge_index, pos, cfg):
    N, NC, LO_LIM = cfg["N"], cfg["NC"], cfg["LO_LIM"]
    NLOC, SLOC, NSLOT, NSLOTP, RTOT, PV = _derived(cfg)
    src = np.asarray(edge_index[0], np.int64)
    dst = np.asarray(edge_index[1], np.int64)
    rows = (src // NLOC) * NSLOTP + (src % NLOC)   # global table row of src
    core_of = dst // NLOC
    dummy_lo = NSLOT                                # core 0 dummy block
    dummy_hi = (NC - 1) * NSLOTP + NSLOT - LO_LIM   # last core dummy block

    sides = [[], []]     # sides[0][c] = lo side of core c
    for c in range(NC):
        m = core_of == c
        s_rows = rows[m]
        d_loc = dst[m] - c * NLOC
        for si, sel in ((0, s_rows < LO_LIM), (1, s_rows >= LO_LIM)):
            s = s_rows[sel] - (0 if si == 0 else LO_LIM)
            d = d_loc[sel]
            deg = np.bincount(d, minlength=NSLOT)
            order = np.argsort(-deg, kind="stable")
            slot_of = np.empty(NSLOT, np.int64)
            slot_of[order] = np.arange(NSLOT)
            isort = np.argsort(d, kind="stable")
            starts = np.zeros(NSLOT + 1, np.int64)
            np.cumsum(deg, out=starts[1:])
            sides[si].append({"deg": deg, "order": order, "slot_of": slot_of,
                              "s_sorted": s[isort], "starts": starts,
                              "cnts": np.sort(deg)[::-1]})

    sched = []
    for si in range(2):
        Sk = []
        kmax = max(int(sd["cnts"][0]) for sd in sides[si])
        for k in range(kmax):
            cnt = max(int((sd["cnts"] > k).sum()) for sd in sides[si])
            if cnt == 0:
                break
            Sk.append(_ceil(cnt, 128))
        sched.append(Sk)

    j = np.arange(NSLOT)
    n_of_j = (j % 128) * SLOC + j // 128

    import ml_dtypes
    per_core = []
    for c in range(NC):
        blocks = {0: [], 1: []}
        for si in range(2):
            sd = sides[si][c]
            dummy = dummy_lo if si == 0 else dummy_hi
            for k, S in enumerate(sched[si]):
                L = S * 128
                ids = np.full(L, dummy, np.int64)
                nsl = int((sd["cnts"] > k).sum())
                nodes = sd["order"][:nsl]
                ids[:nsl] = sd["s_sorted"][sd["starts"][nodes] + k]
                blocks[si].append(_wrap16(ids))
        gi_lo = (np.concatenate(blocks[0], axis=1) if blocks[0]
                 else np.zeros((128, 8), np.int16))
        gi_hi = (np.concatenate(blocks[1], axis=1) if blocks[1]
                 else np.zeros((128, 8), np.int16))
        mg_lo = _wrap16(sides[0][c]["slot_of"][n_of_j])
        mg_hi = _wrap16(sides[1][c]["slot_of"][n_of_j] + NSLOT)
        gnode = np.minimum(c * NLOC + n_of_j, N - 1)
        posm = np.ascontiguousarray(
            np.asarray(pos)[gnode].T).astype(np.float16)
        per_core.append({"gi_lo": gi_lo, "gi_hi": gi_hi, "mg_lo": mg_lo,
                         "mg_hi": mg_hi, "posm": posm})
    return per_core, sched


def build_bass(cfg, sched, reps=1, timeline=False):
    import concourse.bass as bass
    import concourse.bacc as bacc
    import concourse.tile as tile
    from concourse import mybir
    from concourse.masks import make_identity
    import contextlib

    N, NC = cfg["N"], cfg["NC"]
    CIN, COUT = cfg["CIN"], cfg["COUT"]
    LO_LIM = cfg["LO_LIM"]
    NLOC, SLOC, NSLOT, NSLOTP, RTOT, PV = _derived(cfg)
    f32, bf16, i16 = mybir.dt.float32, mybir.dt.float16, mybir.dt.int16
    OP = mybir.AluOpType
    AF = mybir.ActivationFunctionType

    nc = bacc.Bacc(num_devices=(1 if timeline else NC), name="blockconv",
                   dynamic_dma_scratch_size=16384)

    xs_in = nc.dram_tensor("xs", [NLOC, CIN], f32, kind="ExternalInput")
    posm_in = nc.dram_tensor("posm", [3, NSLOT], bf16, kind="ExternalInput")
    wt = {}
    for nm, shp in (("W1", [CIN + 3, COUT]), ("b1", [1, COUT]),
                    ("W2", [COUT + 3, COUT]), ("b2", [1, COUT]),
                    ("Wl", [CIN, COUT]), ("bl", [1, COUT]),
                    ("g1", [COUT, 1]), ("be1", [COUT, 1]), ("g2", [COUT, 1]),
                    ("be2", [COUT, 1]), ("gl", [COUT, 1]), ("bel", [COUT, 1])):
        wt[nm] = nc.dram_tensor(nm, shp, f32, kind="ExternalInput")

    Wlo = max(sum(sched[0]), 1) * 8
    Whi = max(sum(sched[1]), 1) * 8
    gi_lo_in = nc.dram_tensor("gi_lo", [128, Wlo], i16, kind="ExternalInput")
    gi_hi_in = nc.dram_tensor("gi_hi", [128, Whi], i16, kind="ExternalInput")
    mg_lo_in = nc.dram_tensor("mg_lo", [128, NSLOT // 16], i16, kind="ExternalInput")
    mg_hi_in = nc.dram_tensor("mg_hi", [128, NSLOT // 16], i16, kind="ExternalInput")

    out_t = nc.dram_tensor("out", [NSLOT, COUT], f32, kind="ExternalOutput")

    shared = "Local" if timeline else "Shared"
    ag1_i = nc.dram_tensor("ag1_in", [NSLOTP, COUT], bf16)
    ag1_o = nc.dram_tensor("ag1_out", [NC, NSLOTP, COUT], bf16, addr_space=shared)
    ag2_i = nc.dram_tensor("ag2_in", [NSLOTP, COUT], bf16)
    ag2_o = nc.dram_tensor("ag2_out", [NC, NSLOTP, COUT], bf16, addr_space=shared)
    mbuf = nc.dram_tensor("mbuf", [2 * NSLOT, COUT], bf16)
    ar_i = nc.dram_tensor("ar_in", [COUT, 4], f32)
    ar_o = nc.dram_tensor("ar_out", [COUT, 4], f32, addr_space=shared)
    rowbuf = nc.dram_tensor("rowbuf", [6, COUT], f32)
    ar2_i = nc.dram_tensor("ar2_in", [COUT, 2], f32)
    ar2_o = nc.dram_tensor("ar2_out", [COUT, 2], f32, addr_space=shared)
    groups = [list(range(NC))]

    with tile.TileContext(nc) as tc:
        ctx = contextlib.ExitStack()
        with ctx:
            sing = ctx.enter_context(tc.tile_pool(name="sing", bufs=1))
            xp = ctx.enter_context(tc.tile_pool(name="xp", bufs=3))
            pp = ctx.enter_context(tc.tile_pool(name="pp", bufs=2, space="PSUM"))
            pq = ctx.enter_context(tc.tile_pool(name="pq", bufs=2, space="PSUM"))
            pr = ctx.enter_context(tc.tile_pool(name="pr", bufs=1, space="PSUM"))
            cp = ctx.enter_context(tc.tile_pool(name="cp", bufs=4))
            ap_ = ctx.enter_context(tc.tile_pool(name="ap", bufs=1))
            st = ctx.enter_context(tc.tile_pool(name="st", bufs=2))
            sm = ctx.enter_context(tc.tile_pool(name="sm", bufs=2))

            ident = sing.tile([128, 128], f32)
            make_identity(nc, ident)
            identb = sing.tile([128, 128], bf16)
            nc.vector.tensor_copy(out=identb[:], in_=ident[:])
            ones1 = sing.tile([1, 128], f32)
            nc.vector.memset(ones1[:], 1.0)
            ones1v = sing.tile([1, 128], f32)   # valid-partition row mask
            nc.vector.memset(ones1v[:], 1.0)
            if PV < 128:
                nc.vector.memset(ones1v[:, PV:128], 0.0)
            onesp = sing.tile([128, 1], f32)
            nc.vector.memset(onesp[:], 1.0)
            onespb = sing.tile([128, 1], bf16)
            nc.vector.memset(onespb[:], 1.0)
            negbig = sing.tile([128, COUT], bf16)
            nc.vector.memset(negbig[:], BIG_NEG16)
            epsv = sing.tile([COUT, 1], f32)
            nc.vector.memset(epsv[:], EPS)

            W1s = sing.tile([CIN + 3, COUT], f32)
            nc.sync.dma_start(W1s[:], wt["W1"][:])
            W1pb = sing.tile([3, COUT], bf16)
            nc.vector.tensor_copy(out=W1pb[:], in_=W1s[CIN:CIN + 3, :])
            W2af = sing.tile([COUT, COUT], f32)
            nc.sync.dma_start(W2af[:], wt["W2"][0:COUT, :])
            W2ab = sing.tile([COUT, COUT], bf16)
            nc.vector.tensor_copy(out=W2ab[:], in_=W2af[:])
            W2pf = sing.tile([3, COUT], f32)
            nc.sync.dma_start(W2pf[:], wt["W2"][COUT:COUT + 3, :])
            W2pb = sing.tile([3, COUT], bf16)
            nc.vector.tensor_copy(out=W2pb[:], in_=W2pf[:])
            Wlf = sing.tile([CIN, COUT], f32)
            nc.sync.dma_start(Wlf[:], wt["Wl"][:])
            brow = {}
            for nm in ("b1", "b2", "bl"):
                t = sing.tile([1, COUT], f32, tag=f"br_{nm}")
                nc.sync.dma_start(t[:], wt[nm][:])
                brow[nm] = t
            pvec = {}
            for nm in ("g1", "be1", "g2", "be2", "gl", "bel"):
                v = sing.tile([COUT, 1], f32, tag=f"pv_{nm}")
                nc.sync.dma_start(v[:], wt[nm][:])
                pvec[nm] = v

            posmb = sing.tile([3, NSLOT], bf16)
            nc.sync.dma_start(posmb[:], posm_in[:])
            idx_lo = sing.tile([128, Wlo], i16)
            nc.sync.dma_start(idx_lo[:], gi_lo_in[:])
            idx_hi = sing.tile([128, Whi], i16)
            nc.sync.dma_start(idx_hi[:], gi_hi_in[:])
            midx = {}
            for nm, t_ in (("lo", mg_lo_in), ("hi", mg_hi_in)):
                m_ = sing.tile([128, NSLOT // 16], i16, tag=f"mi_{nm}")
                nc.sync.dma_start(m_[:], t_[:])
                midx[nm] = m_

            for _rep in range(reps):
                # -BIG dummy rows NSLOT..NSLOT+127 of this core's slices
                nc.sync.dma_start(ag1_i[NSLOT:NSLOTP, :], negbig[:])
                nc.sync.dma_start(ag2_i[NSLOT:NSLOTP, :], negbig[:])

                # ---------- build a1 + skip in slot layout -------------
                xbig = sing.tile([128, SLOC, CIN], f32, tag="xbig")
                nc.vector.memset(xbig[:], 0.0)
                nc.sync.dma_start(
                    xbig[0:PV], xs_in[:].rearrange("(p s) c -> p s c", s=SLOC))
                a1 = ap_.tile([128, SLOC, COUT], bf16, tag="big1")
                skt = ap_.tile([128, SLOC, COUT], f32, tag="bigsk")
                for s in range(SLOC):
                    ps = pp.tile([128, 128], f32, tag="pst")
                    nc.tensor.transpose(out=ps[0:CIN, :], in_=xbig[:, s, :], identity=ident[:])
                    xT = xp.tile([CIN, 128], f32, tag="xT")
                    nc.scalar.copy(out=xT[:], in_=ps[0:CIN, :])
                    pb = pq.tile([128, COUT], f32, tag="pout")
                    nc.tensor.matmul(out=pb[:], lhsT=xT[:], rhs=W1s[0:CIN, :], start=True, stop=False)
                    nc.tensor.matmul(out=pb[:], lhsT=posmb[:, s * 128:(s + 1) * 128], rhs=W1pb[:], start=False, stop=False)
                    nc.tensor.matmul(out=pb[:], lhsT=ones1[:], rhs=brow["b1"][:], start=False, stop=True)
                    nc.vector.tensor_copy(out=a1[:, s, :], in_=pb[:])
                    pl = pq.tile([128, COUT], f32, tag="pout")
                    nc.tensor.matmul(out=pl[:], lhsT=xT[:], rhs=Wlf[:], start=True, stop=False)
                    nc.tensor.matmul(out=pl[:], lhsT=ones1v[:], rhs=brow["bl"][:], start=False, stop=True)
                    nc.scalar.copy(out=skt[:, s, :], in_=pl[:])

                arst = sing.tile([COUT, 4], f32)

                # PE-chained stats: sums via ones matmul, squares via diag(v^T v)
                def stats_into(vtile, arcols, ones_vec):
                    psum_ = pr.tile([COUT, 1], f32, tag="psum")
                    psq = pr.tile([128, 128], f32, tag="psq")
                    for s in range(SLOC):
                        nc.tensor.matmul(out=psum_[:], lhsT=vtile[:, s, :], rhs=ones_vec[:],
                                         start=(s == 0), stop=(s == SLOC - 1))
                        nc.tensor.matmul(out=psq[:], lhsT=vtile[:, s, :], rhs=vtile[:, s, :],
                                         start=(s == 0), stop=(s == SLOC - 1))
                    sq = sm.tile([128, 128], f32, tag="sqd")
                    nc.vector.tensor_tensor(out=sq[:], in0=psq[:], in1=ident[:], op=OP.mult)
                    nc.vector.tensor_copy(out=arcols[:, 0:1], in_=psum_[:])
                    nc.vector.tensor_reduce(out=arcols[:, 1:2], in_=sq[:],
                                            op=OP.add, axis=mybir.AxisListType.X)

                stats_into(skt, arst[:, 2:4], onesp)
                # a1 -> ag1_i rows 0..NLOC
                nc.sync.dma_start(
                    ag1_i[0:NLOC, :].rearrange("(p s) f -> p s f", s=SLOC), a1[0:PV])

                def allgather(src, dst):
                    if timeline:
                        for q in range(_ceil(NSLOTP, 128)):
                            r0, r1 = q * 128, min((q + 1) * 128, NSLOTP)
                            t_ = cp.tile([128, COUT], bf16, tag="agb")
                            nc.sync.dma_start(t_[0:r1 - r0], src[r0:r1, :])
                            nc.sync.dma_start(dst[0, r0:r1, :], t_[0:r1 - r0])
                    else:
                        nc.gpsimd.collective_compute(
                            "AllGather", OP.bypass, replica_groups=groups,
                            ins=[src[:]], outs=[dst[:]])

                allgather(ag1_i, ag1_o)

                # ---------------- gather-max passes ----------------
                GMAX = 8   # max 8*128=1024 indices per dma_gather (SWDGE ring cap)

                def gather_chunked(dst3, in_ap, idxt, chunk0, nchunks):
                    a = 0
                    while a < nchunks:
                        b = min(a + GMAX, nchunks)
                        nc.gpsimd.dma_gather(
                            out_ap=dst3[:, a:b, :], in_ap=in_ap,
                            idxs_ap=idxt[:, (chunk0 + a) * 8:(chunk0 + b) * 8],
                            num_idxs=(b - a) * 128, num_idxs_reg=(b - a) * 128,
                            elem_size=COUT)
                        a = b

                def gather_conv(ag_o, acc_tags):
                    flat = ag_o[:].rearrange("c n f -> (c n) f")
                    acc = {}
                    for snm, tg in zip(("lo", "hi"), acc_tags):
                        a = ap_.tile([128, SLOC, COUT], bf16, tag=tg)
                        nc.gpsimd.memset(a[:], BIG_NEG16)
                        acc[snm] = a
                    for snm, idxt, w0, w1 in (("lo", idx_lo, 0, LO_LIM),
                                              ("hi", idx_hi, LO_LIM, RTOT)):
                        off = 0
                        for k, S in enumerate(sched[0 if snm == "lo" else 1]):
                            stg = st.tile([128, SLOC, COUT], bf16, tag="stage")
                            gather_chunked(stg[:, 0:S, :], flat[w0:w1, :],
                                           idxt, off // 8, S)
                            nc.vector.tensor_tensor(
                                out=acc[snm][:, 0:S, :], in0=acc[snm][:, 0:S, :],
                                in1=stg[:, 0:S, :], op=OP.max)
                            off += 8 * S
                    # merge via HBM round-trip, node order
                    nc.sync.dma_start(
                        mbuf[0:NSLOT, :].rearrange("(s p) f -> p s f", p=128), acc["lo"][:])
                    nc.sync.dma_start(
                        mbuf[NSLOT:2 * NSLOT, :].rearrange("(s p) f -> p s f", p=128), acc["hi"][:])
                    g1t = st.tile([128, SLOC, COUT], bf16, tag="stage")
                    gather_chunked(g1t[:], mbuf[:], midx["lo"], 0, SLOC)
                    g2t = st.tile([128, SLOC, COUT], bf16, tag="stage")
                    gather_chunked(g2t[:], mbuf[:], midx["hi"], 0, SLOC)
                    agg = ap_.tile([128, SLOC, COUT], bf16, tag=acc_tags[0])
                    nc.vector.tensor_tensor(out=agg[:], in0=g1t[:], in1=g2t[:], op=OP.max)
                    return agg

                agg1 = gather_conv(ag1_o, ("big1", "big2"))

                # v = (agg - pc) * (agg > -1e29), per chunk s
                def v_compute(agg, Wp, vtag, vT=None):
                    v = ap_.tile([128, SLOC, COUT], bf16, tag=vtag)
                    for s in range(SLOC):
                        pc = pq.tile([128, COUT], f32, tag="pout")
                        nc.tensor.matmul(out=pc[:], lhsT=posmb[:, s * 128:(s + 1) * 128],
                                         rhs=Wp[:], start=True, stop=True)
                        msk = cp.tile([128, COUT], bf16, tag="msk")
                        nc.vector.tensor_scalar(out=msk[:], in0=agg[:, s, :], scalar1=-30000.0,
                                                scalar2=None, op0=OP.is_gt)
                        t_ = cp.tile([128, COUT], bf16, tag="tv")
                        nc.vector.tensor_tensor(out=t_[:], in0=agg[:, s, :], in1=pc[:], op=OP.subtract)
                        nc.vector.tensor_tensor(out=v[:, s, :], in0=t_[:], in1=msk[:], op=OP.mult)
                        if vT is not None:
                            pt = pr.tile([128, 128], bf16, tag="psb")
                            nc.tensor.transpose(out=pt[:], in_=v[:, s, :], identity=identb[:])
                            nc.scalar.copy(out=vT[:, s, :], in_=pt[:])
                    return v

                v1T = ap_.tile([128, SLOC, 128], bf16, tag="big3")
                v1 = v_compute(agg1, W1pb, "big2", vT=v1T)
                stats_into(v1, arst[:, 0:2], onespb)
                nc.sync.dma_start(ar_i[:], arst[:])
                if timeline:
                    _t = sm.tile([COUT, 4], f32, tag="cc1")
                    nc.sync.dma_start(_t[:], ar_i[:])
                    nc.sync.dma_start(ar_o[:], _t[:])
                else:
                    nc.gpsimd.collective_compute("AllReduce", OP.add, replica_groups=groups,
                                                 ins=[ar_i[:]], outs=[ar_o[:]])
                arres = sing.tile([COUT, 4], f32, tag="arres")
                nc.sync.dma_start(arres[:], ar_o[:])

                # BN params: scale = g * rsqrt(var+eps), shift = be - mean*scale
                def bn_vecs(sum_ap, sq_ap, g_v, be_v, tagp):
                    mean = sm.tile([COUT, 1], f32, tag=f"{tagp}_m")
                    nc.vector.tensor_scalar(out=mean[:], in0=sum_ap, scalar1=1.0 / N,
                                            scalar2=None, op0=OP.mult)
                    ex2 = sm.tile([COUT, 1], f32, tag=f"{tagp}_e")
                    nc.vector.tensor_scalar(out=ex2[:], in0=sq_ap, scalar1=1.0 / N,
                                            scalar2=None, op0=OP.mult)
                    m2 = sm.tile([COUT, 1], f32, tag=f"{tagp}_m2")
                    nc.vector.tensor_tensor(out=m2[:], in0=mean[:], in1=mean[:], op=OP.mult)
                    var = sm.tile([COUT, 1], f32, tag=f"{tagp}_v")
                    nc.vector.tensor_tensor(out=var[:], in0=ex2[:], in1=m2[:], op=OP.subtract)
                    sd = sm.tile([COUT, 1], f32, tag=f"{tagp}_sd")
                    nc.scalar.activation(out=sd[:], in_=var[:], func=AF.Sqrt, bias=epsv[:], scale=1.0)
                    rstd = sm.tile([COUT, 1], f32, tag=f"{tagp}_r")
                    nc.vector.reciprocal(out=rstd[:], in_=sd[:])
                    ssh = sm.tile([COUT, 2], f32, tag=f"{tagp}_ssh")
                    nc.vector.tensor_tensor(out=ssh[:, 0:1], in0=rstd[:], in1=g_v[:], op=OP.mult)
                    ms = sm.tile([COUT, 1], f32, tag=f"{tagp}_ms")
                    nc.vector.tensor_tensor(out=ms[:], in0=mean[:], in1=ssh[:, 0:1], op=OP.mult)
                    nc.vector.tensor_tensor(out=ssh[:, 1:2], in0=be_v[:], in1=ms[:], op=OP.subtract)
                    return ssh

                def bn_rows(ssh, tagp):
                    # transpose [COUT,2] -> [2, COUT] rows (scale row 0, shift row 1)
                    prow = pr.tile([2, COUT], f32, tag="prow")
                    nc.tensor.transpose(out=prow[:], in_=ssh[:], identity=ident[:])
                    rows = sing.tile([2, COUT], f32, tag=f"{tagp}_rows")
                    nc.vector.tensor_copy(out=rows[:], in_=prow[:])
                    slot = {"bn1": 0, "bnl": 2, "bn2": 4}[tagp]
                    nc.sync.dma_start(rowbuf[slot:slot + 2, :], rows[:])
                    bc = sing.tile([128, 2, COUT], f32, tag=f"{tagp}_bc")
                    rap = rowbuf[slot:slot + 2, :]
                    nc.sync.dma_start(bc[:], bass.AP(tensor=rap.tensor, offset=rap.offset,
                                                     ap=[[0, 128]] + list(rap.ap)))
                    return bc

                ssh1 = bn_vecs(arres[:, 0:1], arres[:, 1:2], pvec["g1"], pvec["be1"], "bn1")
                sshl = bn_vecs(arres[:, 2:3], arres[:, 3:4], pvec["gl"], pvec["bel"], "bnl")
                rowsl = bn_rows(sshl, "bnl")

                # h1T = relu(v1T*scale1 + shift1): one Act op in transposed
                # (feature-major) layout, using the v1T built during the AR
                nc.scalar.activation(out=v1T[:].rearrange("f s n -> f (s n)"),
                                     in_=v1T[:].rearrange("f s n -> f (s n)"),
                                     func=AF.Relu, bias=ssh1[:, 1:2], scale=ssh1[:, 0:1])

                # ---------- a2 = [h1|pos]@W2 + b2 (slot layout) --------
                a2 = ap_.tile([128, SLOC, COUT], bf16, tag="big1")
                for s in range(SLOC):
                    pb = pq.tile([128, COUT], f32, tag="pout")
                    nc.tensor.matmul(out=pb[:], lhsT=v1T[:, s, :], rhs=W2ab[:], start=True, stop=False)
                    nc.tensor.matmul(out=pb[:], lhsT=posmb[:, s * 128:(s + 1) * 128], rhs=W2pb[:], start=False, stop=False)
                    nc.tensor.matmul(out=pb[:], lhsT=ones1[:], rhs=brow["b2"][:], start=False, stop=True)
                    nc.vector.tensor_copy(out=a2[:, s, :], in_=pb[:])
                nc.sync.dma_start(
                    ag2_i[0:NLOC, :].rearrange("(p s) f -> p s f", s=SLOC), a2[0:PV])
                allgather(ag2_i, ag2_o)

                agg2 = gather_conv(ag2_o, ("big1", "big2"))
                v2 = v_compute(agg2, W2pb, "big2")

                arst2 = sing.tile([COUT, 2], f32, tag="arst2")
                stats_into(v2, arst2[:], onespb)
                nc.sync.dma_start(ar2_i[:], arst2[:])
                if timeline:
                    _t = sm.tile([COUT, 2], f32, tag="cc2")
                    nc.sync.dma_start(_t[:], ar2_i[:])
                    nc.sync.dma_start(ar2_o[:], _t[:])
                else:
                    nc.gpsimd.collective_compute("AllReduce", OP.add, replica_groups=groups,
                                                 ins=[ar2_i[:]], outs=[ar2_o[:]])
                arres2 = sing.tile([COUT, 2], f32, tag="arres2")
                nc.sync.dma_start(arres2[:], ar2_o[:])
                rows2 = bn_rows(bn_vecs(arres2[:, 0:1], arres2[:, 1:2], pvec["g2"], pvec["be2"], "bn2"), "bn2")

                # final = relu(bn2(v2) + bnl(skip)); skt transformed in place
                fin = ap_.tile([128, SLOC, COUT], f32, tag="bigf")
                nc.vector.tensor_tensor(out=fin[:], in0=v2[:],
                                        in1=rows2[:, 0:1, :].to_broadcast([128, SLOC, COUT]), op=OP.mult)
                nc.vector.tensor_tensor(out=fin[:], in0=fin[:],
                                        in1=rows2[:, 1:2, :].to_broadcast([128, SLOC, COUT]), op=OP.add)
                nc.vector.tensor_tensor(out=skt[:], in0=skt[:],
                                        in1=rowsl[:, 0:1, :].to_broadcast([128, SLOC, COUT]), op=OP.mult)
                nc.vector.tensor_tensor(out=skt[:], in0=skt[:],
                                        in1=rowsl[:, 1:2, :].to_broadcast([128, SLOC, COUT]), op=OP.add)
                nc.vector.tensor_tensor(out=fin[:], in0=fin[:], in1=skt[:], op=OP.add)
                nc.vector.tensor_scalar(out=fin[:], in0=fin[:], scalar1=0.0, scalar2=None, op0=OP.max)
                nc.sync.dma_start(out_t[:].rearrange("(p s) f -> p s f", p=128), fin[:])

    nc.compile()
    return nc


def make_in_maps(inputs, cfg, per_core):
    N, NC, CIN = cfg["N"], cfg["NC"], cfg["CIN"]
    NLOC = N // NC
    x = np.ascontiguousarray(np.asarray(inputs["x"], np.float32))
    shared = dict(
        W1=np.asarray(inputs["W1"], np.float32),
        b1=np.asarray(inputs["b1"], np.float32).reshape(1, -1),
        W2=np.asarray(inputs["W2"], np.float32),
        b2=np.asarray(inputs["b2"], np.float32).reshape(1, -1),
        Wl=np.asarray(inputs["Wl"], np.float32),
        bl=np.asarray(inputs["bl"], np.float32).reshape(1, -1),
        g1=np.asarray(inputs["g1"], np.float32).reshape(-1, 1),
        be1=np.asarray(inputs["be1"], np.float32).reshape(-1, 1),
        g2=np.asarray(inputs["g2"], np.float32).reshape(-1, 1),
        be2=np.asarray(inputs["be2"], np.float32).reshape(-1, 1),
        gl=np.asarray(inputs["gl"], np.float32).reshape(-1, 1),
        bel=np.asarray(inputs["bel"], np.float32).reshape(-1, 1),
    )
    in_maps = []
    for c in range(NC):
        m = dict(shared)
        m["xs"] = np.ascontiguousarray(x[c * NLOC:(c + 1) * NLOC])
        pc = per_core[c]
        for k in ("gi_lo", "gi_hi", "mg_lo", "mg_hi", "posm"):
            m[k] = pc[k]
        in_maps.append(m)
    return in_maps


_CACHE = {}


def run(inputs, cfg, use_sim=False, trace=False):
    per_core, sched = host_prep(inputs["edge_index"], inputs["pos"], cfg)
    key = (cfg["N"], tuple(sched[0]), tuple(sched[1]))
    if key not in _CACHE:
        _CACHE[key] = build_bass(cfg, sched)
    nc = _CACHE[key]
    in_maps = make_in_maps(inputs, cfg, per_core)
    NC = cfg["NC"]
    NLOC = cfg["N"] // NC
    if use_sim:
        from concourse.bass_interp import MultiCoreSim
        sim = MultiCoreSim(nc, num_cores=NC, require_finite=False, require_nnan=False)
        for c in range(NC):
            for k, v in in_maps[c].items():
                sim.cores[c].tensor(k)[:] = v
        sim.simulate(check_with_hw=False)
        outs = [np.array(sim.cores[c].tensor("out")) for c in range(NC)]
        res = None
    else:
        from concourse.bass_utils import run_bass_kernel_spmd
        res = run_bass_kernel_spmd(nc, in_maps, core_ids=list(range(NC)), trace=trace)
        outs = [res.results[c]["out"] for c in range(NC)]
    full = np.concatenate([o[:NLOC] for o in outs], axis=0)
    return full, res


def kernel(**inputs):
    out, _ = run(inputs, FULL_CFG, use_sim=False)
    return out


# revision 15
# speedup vs baseline: 2502.9055x; 5.3988x over previous
"""Trainium2 Bass kernel for nn_BlockConv (PointNet-style GNN block), 8 cores.

Algebraic core: msg_e = concat(x_src, pos_src-pos_dst) @ W + b
  = A[src] - C[dst], with A = concat(x,pos)@W + b (per-node table) and
  C = pos@W[-3:] (per-dst, constant within a segment). segment_max over
  dst = (gather+max of A rows) - C[dst]. Pure memory problem.

Distribution: dst-sharded; each core computes the A-table rows for ITS
nodes only (slot layout, fp16) and one AllGather per conv materializes
the full gather table in shared HBM (ag_o = [NC, NSLOT+128, COUT]; rows
NSLOT..NSLOT+127 of every slice are -BIG dummies for pass padding).
Global row of node n = (n//NLOC)*(NSLOT+128) + n%NLOC. Skip path is
computed in slot layout from a strided x read (no regather); BN stats
are chained PE matmuls (sum via ones vector, sum-of-squares via v^T v
diagonal), AllReduced across cores.

Gather: dma_gather (int16 idx) over lo/hi row windows split at 32768;
per window the core's nodes are degree-sorted so pass k covers a slot
prefix; DVE max chains accumulate; one HBM round-trip re-gathers both
accumulators in node order and maxes them.
"""
import sys
import numpy as np

if "/opt/trn_rl_repo" not in sys.path:
    sys.path.insert(0, "/opt/trn_rl_repo")

BIG_NEG = -1.0e30
BIG_NEG16 = -60000.0
EPS = 1e-5

FULL_CFG = dict(N=50000, E=800000, CIN=64, COUT=128, NC=8, LO_LIM=32768)
MINI_CFG = dict(N=2048, E=16384, CIN=64, COUT=128, NC=8, LO_LIM=1024)
MID_CFG = dict(N=16384, E=262144, CIN=64, COUT=128, NC=8, LO_LIM=8192)


def _ceil(a, b):
    return (a + b - 1) // b


def _derived(cfg):
    N, NC = cfg["N"], cfg["NC"]
    NLOC = N // NC
    SLOC = _ceil(NLOC, 128)       # smallest divisor of NLOC >= ceil(NLOC/128)
    while NLOC % SLOC:
        SLOC += 1
    NSLOT = SLOC * 128
    NSLOTP = NSLOT + 128          # +128 dummy -BIG rows per core slice
    RTOT = NC * NSLOTP
    PV = min(128, NLOC // SLOC)   # valid partitions (slots p*SLOC+s < NLOC)
    return NLOC, SLOC, NSLOT, NSLOTP, RTOT, PV


def _wrap16(ids):
    """flat int list (len % 128 == 0) -> [128, len//16] int16 wrapped:
    unwrapped[j] = g[j%16, j//16], replicated over the 8 core groups."""
    a = np.asarray(ids, np.int64)
    assert a.size % 128 == 0 and a.min() >= 0 and a.max() < 32768
    g = a.reshape(a.size // 16, 16).T.astype(np.int16)   # [16, L/16]
    return np.tile(g, (8, 1))                            # [128, L/16]


def host_prep(edge_index, pos, cfg):
    N, NC, LO_LIM = cfg["N"], cfg["NC"], cfg["LO_LIM"]
    NLOC, SLOC, NSLOT, NSLOTP, RTOT, PV = _derived(cfg)
    src = np.asarray(edge_index[0], np.int64)
    dst = np.asarray(edge_index[1], np.int64)
    rows = (src // NLOC) * NSLOTP + (src % NLOC)   # global table row of src
    core_of = dst // NLOC
    dummy_lo = NSLOT                                # core 0 dummy block
    dummy_hi = (NC - 1) * NSLOTP + NSLOT - LO_LIM   # last core dummy block

    sides = [[], []]     # sides[0][c] = lo side of core c
    for c in range(NC):
        m = core_of == c
        s_rows = rows[m]
        d_loc = dst[m] - c * NLOC
        for si, sel in ((0, s_rows < LO_LIM), (1, s_rows >= LO_LIM)):
            s = s_rows[sel] - (0 if si == 0 else LO_LIM)
            d = d_loc[sel]
            deg = np.bincount(d, minlength=NSLOT)
            order = np.argsort(-deg, kind="stable")
            slot_of = np.empty(NSLOT, np.int64)
            slot_of[order] = np.arange(NSLOT)
            isort = np.argsort(d, kind="stable")
            starts = np.zeros(NSLOT + 1, np.int64)
            np.cumsum(deg, out=starts[1:])
            sides[si].append({"deg": deg, "order": order, "slot_of": slot_of,
                              "s_sorted": s[isort], "starts": starts,
                              "cnts": np.sort(deg)[::-1]})

    sched = []
    for si in range(2):
        Sk = []
        kmax = max(int(sd["cnts"][0]) for sd in sides[si])
        for k in range(kmax):
            cnt = max(int((sd["cnts"] > k).sum()) for sd in sides[si])
            if cnt == 0:
                break
            Sk.append(_ceil(cnt, 128))
        sched.append(Sk)

    j = np.arange(NSLOT)
    n_of_j = (j % 128) * SLOC + j // 128

    import ml_dtypes
    per_core = []
    for c in range(NC):
        blocks = {0: [], 1: []}
        for si in range(2):
            sd = sides[si][c]
            dummy = dummy_lo if si == 0 else dummy_hi
            for k, S in enumerate(sched[si]):
                L = S * 128
                ids = np.full(L, dummy, np.int64)
                nsl = int((sd["cnts"] > k).sum())
                nodes = sd["order"][:nsl]
                ids[:nsl] = sd["s_sorted"][sd["starts"][nodes] + k]
                blocks[si].append(_wrap16(ids))
        gi_lo = (np.concatenate(blocks[0], axis=1) if blocks[0]
                 else np.zeros((128, 8), np.int16))
        gi_hi = (np.concatenate(blocks[1], axis=1) if blocks[1]
                 else np.zeros((128, 8), np.int16))
        mg_lo = _wrap16(sides[0][c]["slot_of"][n_of_j])
        mg_hi = _wrap16(sides[1][c]["slot_of"][n_of_j] + NSLOT)
        gnode = np.minimum(c * NLOC + n_of_j, N - 1)
        posm = np.ascontiguousarray(
            np.asarray(pos)[gnode].T).astype(np.float16)
        per_core.append({"gi_lo": gi_lo, "gi_hi": gi_hi, "mg_lo": mg_lo,
                         "mg_hi": mg_hi, "posm": posm})
    return per_core, sched


def build_bass(cfg, sched, reps=1, timeline=False, no_cc=False):
    import concourse.bass as bass
    import concourse.bacc as bacc
    import concourse.tile as tile
    from concourse import mybir
    from concourse.masks import make_identity
    import contextlib

    N, NC = cfg["N"], cfg["NC"]
    CIN, COUT = cfg["CIN"], cfg["COUT"]
    LO_LIM = cfg["LO_LIM"]
    NLOC, SLOC, NSLOT, NSLOTP, RTOT, PV = _derived(cfg)
    f32, bf16, i16 = mybir.dt.float32, mybir.dt.float16, mybir.dt.int16
    OP = mybir.AluOpType
    AF = mybir.ActivationFunctionType

    nc = bacc.Bacc(num_devices=(1 if timeline else NC), name="blockconv",
                   dynamic_dma_scratch_size=16384, num_swdge_queues=4)

    xs_in = nc.dram_tensor("xs", [NLOC, CIN], f32, kind="ExternalInput")
    posm_in = nc.dram_tensor("posm", [3, NSLOT], bf16, kind="ExternalInput")
    wt = {}
    for nm, shp in (("W1", [CIN + 3, COUT]), ("b1", [1, COUT]),
                    ("W2", [COUT + 3, COUT]), ("b2", [1, COUT]),
                    ("Wl", [CIN, COUT]), ("bl", [1, COUT]),
                    ("g1", [COUT, 1]), ("be1", [COUT, 1]), ("g2", [COUT, 1]),
                    ("be2", [COUT, 1]), ("gl", [COUT, 1]), ("bel", [COUT, 1])):
        wt[nm] = nc.dram_tensor(nm, shp, f32, kind="ExternalInput")

    Wlo = max(sum(sched[0]), 1) * 8
    Whi = max(sum(sched[1]), 1) * 8
    gi_lo_in = nc.dram_tensor("gi_lo", [128, Wlo], i16, kind="ExternalInput")
    gi_hi_in = nc.dram_tensor("gi_hi", [128, Whi], i16, kind="ExternalInput")
    mg_lo_in = nc.dram_tensor("mg_lo", [128, NSLOT // 16], i16, kind="ExternalInput")
    mg_hi_in = nc.dram_tensor("mg_hi", [128, NSLOT // 16], i16, kind="ExternalInput")

    out_t = nc.dram_tensor("out", [NSLOT, COUT], f32, kind="ExternalOutput")

    shared = "Local" if timeline else "Shared"
    ag1_i = nc.dram_tensor("ag1_in", [NSLOTP, COUT], bf16)
    ag1_o = nc.dram_tensor("ag1_out", [NC, NSLOTP, COUT], bf16, addr_space=shared)
    ag2_i = nc.dram_tensor("ag2_in", [NSLOTP, COUT], bf16)
    ag2_o = nc.dram_tensor("ag2_out", [NC, NSLOTP, COUT], bf16, addr_space=shared)
    mbuf = nc.dram_tensor("mbuf", [2 * NSLOT, COUT], bf16)
    ar_i = nc.dram_tensor("ar_in", [COUT, 4], f32)
    ar_o = nc.dram_tensor("ar_out", [COUT, 4], f32, addr_space=shared)
    rowbuf = nc.dram_tensor("rowbuf", [6, COUT], f32)
    ar2_i = nc.dram_tensor("ar2_in", [COUT, 2], f32)
    ar2_o = nc.dram_tensor("ar2_out", [COUT, 2], f32, addr_space=shared)
    groups = [list(range(NC))]

    with tile.TileContext(nc) as tc:
        ctx = contextlib.ExitStack()
        with ctx:
            sing = ctx.enter_context(tc.tile_pool(name="sing", bufs=1))
            xp = ctx.enter_context(tc.tile_pool(name="xp", bufs=3))
            pp = ctx.enter_context(tc.tile_pool(name="pp", bufs=2, space="PSUM"))
            pq = ctx.enter_context(tc.tile_pool(name="pq", bufs=2, space="PSUM"))
            pr = ctx.enter_context(tc.tile_pool(name="pr", bufs=1, space="PSUM"))
            cp = ctx.enter_context(tc.tile_pool(name="cp", bufs=4))
            ap_ = ctx.enter_context(tc.tile_pool(name="ap", bufs=1))
            st = ctx.enter_context(tc.tile_pool(name="st", bufs=2))
            sm = ctx.enter_context(tc.tile_pool(name="sm", bufs=2))

            ident = sing.tile([128, 128], f32)
            make_identity(nc, ident)
            identb = sing.tile([128, 128], bf16)
            nc.vector.tensor_copy(out=identb[:], in_=ident[:])
            ones1 = sing.tile([1, 128], f32)
            nc.vector.memset(ones1[:], 1.0)
            ones1v = sing.tile([1, 128], f32)   # valid-partition row mask
            nc.vector.memset(ones1v[:], 1.0)
            if PV < 128:
                nc.vector.memset(ones1v[:, PV:128], 0.0)
            onesp = sing.tile([128, 1], f32)
            nc.vector.memset(onesp[:], 1.0)
            onespb = sing.tile([128, 1], bf16)
            nc.vector.memset(onespb[:], 1.0)
            negbig = sing.tile([128, COUT], bf16)
            nc.vector.memset(negbig[:], BIG_NEG16)
            epsv = sing.tile([COUT, 1], f32)
            nc.vector.memset(epsv[:], EPS)

            W1s = sing.tile([CIN + 3, COUT], f32)
            nc.sync.dma_start(W1s[:], wt["W1"][:])
            W1pb = sing.tile([3, COUT], bf16)
            nc.vector.tensor_copy(out=W1pb[:], in_=W1s[CIN:CIN + 3, :])
            W2af = sing.tile([COUT, COUT], f32)
            nc.sync.dma_start(W2af[:], wt["W2"][0:COUT, :])
            W2ab = sing.tile([COUT, COUT], bf16)
            nc.vector.tensor_copy(out=W2ab[:], in_=W2af[:])
            W2pf = sing.tile([3, COUT], f32)
            nc.sync.dma_start(W2pf[:], wt["W2"][COUT:COUT + 3, :])
            W2pb = sing.tile([3, COUT], bf16)
            nc.vector.tensor_copy(out=W2pb[:], in_=W2pf[:])
            Wlf = sing.tile([CIN, COUT], f32)
            nc.sync.dma_start(Wlf[:], wt["Wl"][:])
            brow = {}
            for nm in ("b1", "b2", "bl"):
                t = sing.tile([1, COUT], f32, tag=f"br_{nm}")
                nc.sync.dma_start(t[:], wt[nm][:])
                brow[nm] = t
            pvec = {}
            for nm in ("g1", "be1", "g2", "be2", "gl", "bel"):
                v = sing.tile([COUT, 1], f32, tag=f"pv_{nm}")
                nc.sync.dma_start(v[:], wt[nm][:])
                pvec[nm] = v

            posmb = sing.tile([3, NSLOT], bf16)
            nc.sync.dma_start(posmb[:], posm_in[:])
            idx_lo = sing.tile([128, Wlo], i16)
            nc.sync.dma_start(idx_lo[:], gi_lo_in[:])
            idx_hi = sing.tile([128, Whi], i16)
            nc.sync.dma_start(idx_hi[:], gi_hi_in[:])
            midx = {}
            for nm, t_ in (("lo", mg_lo_in), ("hi", mg_hi_in)):
                m_ = sing.tile([128, NSLOT // 16], i16, tag=f"mi_{nm}")
                nc.sync.dma_start(m_[:], t_[:])
                midx[nm] = m_

            for _rep in range(reps):
                # -BIG dummy rows NSLOT..NSLOT+127 of this core's slices
                nc.sync.dma_start(ag1_i[NSLOT:NSLOTP, :], negbig[:])
                nc.sync.dma_start(ag2_i[NSLOT:NSLOTP, :], negbig[:])

                # ---------- build a1 + skip in slot layout -------------
                xbig = sing.tile([128, SLOC, CIN], f32, tag="xbig")
                nc.vector.memset(xbig[:], 0.0)
                nc.sync.dma_start(
                    xbig[0:PV], xs_in[:].rearrange("(p s) c -> p s c", s=SLOC))
                a1 = ap_.tile([128, SLOC, COUT], bf16, tag="big1")
                skt = ap_.tile([128, SLOC, COUT], f32, tag="bigsk")
                for s in range(SLOC):
                    ps = pp.tile([128, 128], f32, tag="pst")
                    nc.tensor.transpose(out=ps[0:CIN, :], in_=xbig[:, s, :], identity=ident[:])
                    xT = xp.tile([CIN, 128], f32, tag="xT")
                    nc.scalar.copy(out=xT[:], in_=ps[0:CIN, :])
                    pb = pq.tile([128, COUT], f32, tag="pout")
                    nc.tensor.matmul(out=pb[:], lhsT=xT[:], rhs=W1s[0:CIN, :], start=True, stop=False)
                    nc.tensor.matmul(out=pb[:], lhsT=posmb[:, s * 128:(s + 1) * 128], rhs=W1pb[:], start=False, stop=False)
                    nc.tensor.matmul(out=pb[:], lhsT=ones1[:], rhs=brow["b1"][:], start=False, stop=True)
                    nc.vector.tensor_copy(out=a1[:, s, :], in_=pb[:])
                    pl = pq.tile([128, COUT], f32, tag="pout")
                    nc.tensor.matmul(out=pl[:], lhsT=xT[:], rhs=Wlf[:], start=True, stop=False)
                    nc.tensor.matmul(out=pl[:], lhsT=ones1v[:], rhs=brow["bl"][:], start=False, stop=True)
                    nc.scalar.copy(out=skt[:, s, :], in_=pl[:])

                arst = sing.tile([COUT, 4], f32)

                # PE-chained stats: sums via ones matmul, squares via diag(v^T v)
                def stats_into(vtile, arcols, ones_vec):
                    psum_ = pr.tile([COUT, 1], f32, tag="psum")
                    psq = pr.tile([128, 128], f32, tag="psq")
                    for s in range(SLOC):
                        nc.tensor.matmul(out=psum_[:], lhsT=vtile[:, s, :], rhs=ones_vec[:],
                                         start=(s == 0), stop=(s == SLOC - 1))
                        nc.tensor.matmul(out=psq[:], lhsT=vtile[:, s, :], rhs=vtile[:, s, :],
                                         start=(s == 0), stop=(s == SLOC - 1))
                    sq = sm.tile([128, 128], f32, tag="sqd")
                    nc.vector.tensor_tensor(out=sq[:], in0=psq[:], in1=ident[:], op=OP.mult)
                    nc.vector.tensor_copy(out=arcols[:, 0:1], in_=psum_[:])
                    nc.vector.tensor_reduce(out=arcols[:, 1:2], in_=sq[:],
                                            op=OP.add, axis=mybir.AxisListType.X)

                stats_into(skt, arst[:, 2:4], onesp)
                # a1 -> ag1_i rows 0..NLOC
                nc.sync.dma_start(
                    ag1_i[0:NLOC, :].rearrange("(p s) f -> p s f", s=SLOC), a1[0:PV])

                def allgather(src, dst):
                    if no_cc:
                        return
                    if timeline:
                        for q in range(_ceil(NSLOTP, 128)):
                            r0, r1 = q * 128, min((q + 1) * 128, NSLOTP)
                            t_ = cp.tile([128, COUT], bf16, tag="agb")
                            nc.sync.dma_start(t_[0:r1 - r0], src[r0:r1, :])
                            nc.sync.dma_start(dst[0, r0:r1, :], t_[0:r1 - r0])
                    else:
                        nc.gpsimd.collective_compute(
                            "AllGather", OP.bypass, replica_groups=groups,
                            ins=[src[:]], outs=[dst[:]])

                allgather(ag1_i, ag1_o)

                # ---------------- gather-max passes ----------------
                GMAX = 8   # max 8*128=1024 indices per dma_gather (SWDGE ring cap)

                qrr = [0]   # round-robin over the 4 SWDGE queues

                def gather_chunked(dst3, in_ap, idxt, chunk0, nchunks):
                    a = 0
                    while a < nchunks:
                        b = min(a + GMAX, nchunks)
                        nc.gpsimd.dma_gather(
                            out_ap=dst3[:, a:b, :], in_ap=in_ap,
                            idxs_ap=idxt[:, (chunk0 + a) * 8:(chunk0 + b) * 8],
                            num_idxs=(b - a) * 128, num_idxs_reg=(b - a) * 128,
                            elem_size=COUT, queue_num=qrr[0] % 4)
                        qrr[0] += 1
                        a = b

                def gather_conv(ag_o, acc_tags):
                    flat = ag_o[:].rearrange("c n f -> (c n) f")
                    acc = {}
                    for snm, tg in zip(("lo", "hi"), acc_tags):
                        a = ap_.tile([128, SLOC, COUT], bf16, tag=tg)
                        nc.gpsimd.memset(a[:], BIG_NEG16)
                        acc[snm] = a
                    for snm, idxt, w0, w1 in (("lo", idx_lo, 0, LO_LIM),
                                              ("hi", idx_hi, LO_LIM, RTOT)):
                        off = 0
                        for k, S in enumerate(sched[0 if snm == "lo" else 1]):
                            stg = st.tile([128, SLOC, COUT], bf16, tag="stage")
                            gather_chunked(stg[:, 0:S, :], flat[w0:w1, :],
                                           idxt, off // 8, S)
                            nc.vector.tensor_tensor(
                                out=acc[snm][:, 0:S, :], in0=acc[snm][:, 0:S, :],
                                in1=stg[:, 0:S, :], op=OP.max)
                            off += 8 * S
                    # merge via HBM round-trip, node order
                    nc.sync.dma_start(
                        mbuf[0:NSLOT, :].rearrange("(s p) f -> p s f", p=128), acc["lo"][:])
                    nc.sync.dma_start(
                        mbuf[NSLOT:2 * NSLOT, :].rearrange("(s p) f -> p s f", p=128), acc["hi"][:])
                    g1t = st.tile([128, SLOC, COUT], bf16, tag="stage")
                    gather_chunked(g1t[:], mbuf[:], midx["lo"], 0, SLOC)
                    g2t = st.tile([128, SLOC, COUT], bf16, tag="stage")
                    gather_chunked(g2t[:], mbuf[:], midx["hi"], 0, SLOC)
                    agg = ap_.tile([128, SLOC, COUT], bf16, tag=acc_tags[0])
                    nc.vector.tensor_tensor(out=agg[:], in0=g1t[:], in1=g2t[:], op=OP.max)
                    return agg

                agg1 = gather_conv(ag1_o, ("big1", "big2"))

                # v = (agg - pc) * (agg > -1e29), per chunk s
                def v_compute(agg, Wp, vtag, vT=None):
                    v = ap_.tile([128, SLOC, COUT], bf16, tag=vtag)
                    for s in range(SLOC):
                        pc = pq.tile([128, COUT], f32, tag="pout")
                        nc.tensor.matmul(out=pc[:], lhsT=posmb[:, s * 128:(s + 1) * 128],
                                         rhs=Wp[:], start=True, stop=True)
                        msk = cp.tile([128, COUT], bf16, tag="msk")
                        nc.vector.tensor_scalar(out=msk[:], in0=agg[:, s, :], scalar1=-30000.0,
                                                scalar2=None, op0=OP.is_gt)
                        t_ = cp.tile([128, COUT], bf16, tag="tv")
                        nc.vector.tensor_tensor(out=t_[:], in0=agg[:, s, :], in1=pc[:], op=OP.subtract)
                        nc.vector.tensor_tensor(out=v[:, s, :], in0=t_[:], in1=msk[:], op=OP.mult)
                        if vT is not None:
                            pt = pr.tile([128, 128], bf16, tag="psb")
                            nc.tensor.transpose(out=pt[:], in_=v[:, s, :], identity=identb[:])
                            nc.scalar.copy(out=vT[:, s, :], in_=pt[:])
                    return v

                v1T = ap_.tile([128, SLOC, 128], bf16, tag="big3")
                v1 = v_compute(agg1, W1pb, "big2", vT=v1T)
                stats_into(v1, arst[:, 0:2], onespb)
                nc.sync.dma_start(ar_i[:], arst[:])
                if no_cc:
                    pass
                elif timeline:
                    _t = sm.tile([COUT, 4], f32, tag="cc1")
                    nc.sync.dma_start(_t[:], ar_i[:])
                    nc.sync.dma_start(ar_o[:], _t[:])
                else:
                    nc.gpsimd.collective_compute("AllReduce", OP.add, replica_groups=groups,
                                                 ins=[ar_i[:]], outs=[ar_o[:]])
                arres = sing.tile([COUT, 4], f32, tag="arres")
                nc.sync.dma_start(arres[:], ar_o[:])

                # BN params: scale = g * rsqrt(var+eps), shift = be - mean*scale
                def bn_vecs(sum_ap, sq_ap, g_v, be_v, tagp):
                    mean = sm.tile([COUT, 1], f32, tag=f"{tagp}_m")
                    nc.vector.tensor_scalar(out=mean[:], in0=sum_ap, scalar1=1.0 / N,
                                            scalar2=None, op0=OP.mult)
                    ex2 = sm.tile([COUT, 1], f32, tag=f"{tagp}_e")
                    nc.vector.tensor_scalar(out=ex2[:], in0=sq_ap, scalar1=1.0 / N,
                                            scalar2=None, op0=OP.mult)
                    m2 = sm.tile([COUT, 1], f32, tag=f"{tagp}_m2")
                    nc.vector.tensor_tensor(out=m2[:], in0=mean[:], in1=mean[:], op=OP.mult)
                    var = sm.tile([COUT, 1], f32, tag=f"{tagp}_v")
                    nc.vector.tensor_tensor(out=var[:], in0=ex2[:], in1=m2[:], op=OP.subtract)
                    sd = sm.tile([COUT, 1], f32, tag=f"{tagp}_sd")
                    nc.scalar.activation(out=sd[:], in_=var[:], func=AF.Sqrt, bias=epsv[:], scale=1.0)
                    rstd = sm.tile([COUT, 1], f32, tag=f"{tagp}_r")
                    nc.vector.reciprocal(out=rstd[:], in_=sd[:])
                    ssh = sm.tile([COUT, 2], f32, tag=f"{tagp}_ssh")
                    nc.vector.tensor_tensor(out=ssh[:, 0:1], in0=rstd[:], in1=g_v[:], op=OP.mult)
                    ms = sm.tile([COUT, 1], f32, tag=f"{tagp}_ms")
                    nc.vector.tensor_tensor(out=ms[:], in0=mean[:], in1=ssh[:, 0:1], op=OP.mult)
                    nc.vector.tensor_tensor(out=ssh[:, 1:2], in0=be_v[:], in1=ms[:], op=OP.subtract)
                    return ssh

                def bn_rows(ssh, tagp):
                    # transpose [COUT,2] -> [2, COUT] rows (scale row 0, shift row 1)
                    prow = pr.tile([2, COUT], f32, tag="prow")
                    nc.tensor.transpose(out=prow[:], in_=ssh[:], identity=ident[:])
                    rows = sing.tile([2, COUT], f32, tag=f"{tagp}_rows")
                    nc.vector.tensor_copy(out=rows[:], in_=prow[:])
                    slot = {"bn1": 0, "bnl": 2, "bn2": 4}[tagp]
                    nc.sync.dma_start(rowbuf[slot:slot + 2, :], rows[:])
                    bc = sing.tile([128, 2, COUT], f32, tag=f"{tagp}_bc")
                    rap = rowbuf[slot:slot + 2, :]
                    nc.sync.dma_start(bc[:], bass.AP(tensor=rap.tensor, offset=rap.offset,
                                                     ap=[[0, 128]] + list(rap.ap)))
                    return bc

                ssh1 = bn_vecs(arres[:, 0:1], arres[:, 1:2], pvec["g1"], pvec["be1"], "bn1")
                sshl = bn_vecs(arres[:, 2:3], arres[:, 3:4], pvec["gl"], pvec["bel"], "bnl")
                rowsl = bn_rows(sshl, "bnl")

                # h1T = relu(v1T*scale1 + shift1): one Act op in transposed
                # (feature-major) layout, using the v1T built during the AR
                nc.scalar.activation(out=v1T[:].rearrange("f s n -> f (s n)"),
                                     in_=v1T[:].rearrange("f s n -> f (s n)"),
                                     func=AF.Relu, bias=ssh1[:, 1:2], scale=ssh1[:, 0:1])

                # ---------- a2 = [h1|pos]@W2 + b2 (slot layout) --------
                a2 = ap_.tile([128, SLOC, COUT], bf16, tag="big1")
                for s in range(SLOC):
                    pb = pq.tile([128, COUT], f32, tag="pout")
                    nc.tensor.matmul(out=pb[:], lhsT=v1T[:, s, :], rhs=W2ab[:], start=True, stop=False)
                    nc.tensor.matmul(out=pb[:], lhsT=posmb[:, s * 128:(s + 1) * 128], rhs=W2pb[:], start=False, stop=False)
                    nc.tensor.matmul(out=pb[:], lhsT=ones1[:], rhs=brow["b2"][:], start=False, stop=True)
                    nc.vector.tensor_copy(out=a2[:, s, :], in_=pb[:])
                nc.sync.dma_start(
                    ag2_i[0:NLOC, :].rearrange("(p s) f -> p s f", s=SLOC), a2[0:PV])
                allgather(ag2_i, ag2_o)

                agg2 = gather_conv(ag2_o, ("big1", "big2"))
                v2 = v_compute(agg2, W2pb, "big2")

                arst2 = sing.tile([COUT, 2], f32, tag="arst2")
                stats_into(v2, arst2[:], onespb)
                nc.sync.dma_start(ar2_i[:], arst2[:])
                if no_cc:
                    pass
                elif timeline:
                    _t = sm.tile([COUT, 2], f32, tag="cc2")
                    nc.sync.dma_start(_t[:], ar2_i[:])
                    nc.sync.dma_start(ar2_o[:], _t[:])
                else:
                    nc.gpsimd.collective_compute("AllReduce", OP.add, replica_groups=groups,
                                                 ins=[ar2_i[:]], outs=[ar2_o[:]])
                arres2 = sing.tile([COUT, 2], f32, tag="arres2")
                nc.sync.dma_start(arres2[:], ar2_o[:])
                rows2 = bn_rows(bn_vecs(arres2[:, 0:1], arres2[:, 1:2], pvec["g2"], pvec["be2"], "bn2"), "bn2")

                # final = relu(bn2(v2) + bnl(skip)); skt transformed in place
                fin = ap_.tile([128, SLOC, COUT], f32, tag="bigf")
                nc.vector.tensor_tensor(out=fin[:], in0=v2[:],
                                        in1=rows2[:, 0:1, :].to_broadcast([128, SLOC, COUT]), op=OP.mult)
                nc.vector.tensor_tensor(out=fin[:], in0=fin[:],
                                        in1=rows2[:, 1:2, :].to_broadcast([128, SLOC, COUT]), op=OP.add)
                nc.vector.tensor_tensor(out=skt[:], in0=skt[:],
                                        in1=rowsl[:, 0:1, :].to_broadcast([128, SLOC, COUT]), op=OP.mult)
                nc.vector.tensor_tensor(out=skt[:], in0=skt[:],
                                        in1=rowsl[:, 1:2, :].to_broadcast([128, SLOC, COUT]), op=OP.add)
                nc.vector.tensor_tensor(out=fin[:], in0=fin[:], in1=skt[:], op=OP.add)
                nc.vector.tensor_scalar(out=fin[:], in0=fin[:], scalar1=0.0, scalar2=None, op0=OP.max)
                nc.sync.dma_start(out_t[:].rearrange("(p s) f -> p s f", p=128), fin[:])

    nc.compile()
    return nc


def make_in_maps(inputs, cfg, per_core):
    N, NC, CIN = cfg["N"], cfg["NC"], cfg["CIN"]
    NLOC = N // NC
    x = np.ascontiguousarray(np.asarray(inputs["x"], np.float32))
    shared = dict(
        W1=np.asarray(inputs["W1"], np.float32),
        b1=np.asarray(inputs["b1"], np.float32).reshape(1, -1),
        W2=np.asarray(inputs["W2"], np.float32),
        b2=np.asarray(inputs["b2"], np.float32).reshape(1, -1),
        Wl=np.asarray(inputs["Wl"], np.float32),
        bl=np.asarray(inputs["bl"], np.float32).reshape(1, -1),
        g1=np.asarray(inputs["g1"], np.float32).reshape(-1, 1),
        be1=np.asarray(inputs["be1"], np.float32).reshape(-1, 1),
        g2=np.asarray(inputs["g2"], np.float32).reshape(-1, 1),
        be2=np.asarray(inputs["be2"], np.float32).reshape(-1, 1),
        gl=np.asarray(inputs["gl"], np.float32).reshape(-1, 1),
        bel=np.asarray(inputs["bel"], np.float32).reshape(-1, 1),
    )
    in_maps = []
    for c in range(NC):
        m = dict(shared)
        m["xs"] = np.ascontiguousarray(x[c * NLOC:(c + 1) * NLOC])
        pc = per_core[c]
        for k in ("gi_lo", "gi_hi", "mg_lo", "mg_hi", "posm"):
            m[k] = pc[k]
        in_maps.append(m)
    return in_maps


_CACHE = {}


def run(inputs, cfg, use_sim=False, trace=False):
    per_core, sched = host_prep(inputs["edge_index"], inputs["pos"], cfg)
    key = (cfg["N"], tuple(sched[0]), tuple(sched[1]))
    if key not in _CACHE:
        _CACHE[key] = build_bass(cfg, sched)
    nc = _CACHE[key]
    in_maps = make_in_maps(inputs, cfg, per_core)
    NC = cfg["NC"]
    NLOC = cfg["N"] // NC
    if use_sim:
        from concourse.bass_interp import MultiCoreSim
        sim = MultiCoreSim(nc, num_cores=NC, require_finite=False, require_nnan=False)
        for c in range(NC):
            for k, v in in_maps[c].items():
                sim.cores[c].tensor(k)[:] = v
        sim.simulate(check_with_hw=False)
        outs = [np.array(sim.cores[c].tensor("out")) for c in range(NC)]
        res = None
    else:
        from concourse.bass_utils import run_bass_kernel_spmd
        res = run_bass_kernel_spmd(nc, in_maps, core_ids=list(range(NC)), trace=trace)
        outs = [res.results[c]["out"] for c in range(NC)]
    full = np.concatenate([o[:NLOC] for o in outs], axis=0)
    return full, res


def kernel(**inputs):
    out, _ = run(inputs, FULL_CFG, use_sim=False)
    return out
